# revision 15
# baseline (speedup 1.0000x reference)
"""Trainium2 Bass kernel for nn_BaseBackbone_78194174591299 (ViT + top-1 routing).

Sharding: data-parallel over batch — 8 samples per core x 8 NeuronCores.

Key identity: in the reference's masked dispatch, rows with sel==i still hold
mid when block i runs, so block(t,i)[sel==i] == block(mid,i)[sel==i]. Each
late block (9..11) is computed once on mid, serving both the dispatch (via a
one-hot blend) and the cosine stats.

Device program per core (token-major fp32 residual, bf16 matmul operands):
  patch-embed (im2col DMA + transpose-via-identity-matmul + matmul) -> t
  9 dense blocks (in-place residual)
  router MLP (fp32) -> logits z -> one-hot masks (exact 0/1) + sigmoid pro
  3 late blocks on mid -> blend into acc + cosine stats
  final layernorm -> out

Self-contained: all shapes hardcoded; nothing read from /root/problem.
"""
import numpy as np

B = 64
NCORES = 8
SPC = B // NCORES       # 8 samples per core
L = 320
D = 384
H = 6
DH = D // H             # 64
NL = 12
START = 8
T = SPC * L             # 2560
NTT = T // 128          # 20
LN_EPS = 1e-6
ATT_SCALE = DH ** -0.5

_CACHE = {}
LAST_EXEC_NS = None
LAST_SCOPES = None


def _build(ndense, nlate, zero_bias, trivial_norm):
    from contextlib import ExitStack
    import concourse.bass as bass
    import concourse.tile as tile
    from concourse import bacc, mybir
    from concourse.masks import make_identity

    f32 = mybir.dt.float32
    bf16 = mybir.dt.bfloat16
    AF = mybir.ActivationFunctionType
    ALU = mybir.AluOpType
    AP = bass.AP

    nc = bacc.Bacc("TRN2", target_bir_lowering=False, debug=False)
    dp = nc.declare_dram_parameter

    patches_d = dp("patches", [T, 768], bf16, isOutput=False)
    convw_d = dp("convw", [768, D], bf16, isOutput=False)
    pos_d = dp("pos", [T, D], f32, isOutput=False)
    wqkv_d = dp("wqkv", [NL, D, 3 * D], bf16, isOutput=False)
    wproj_d = dp("wproj", [NL, D, D], bf16, isOutput=False)
    wfc1_d = dp("wfc1", [NL, D, 4 * D], bf16, isOutput=False)
    wfc2_d = dp("wfc2", [NL, 4 * D, D], bf16, isOutput=False)
    bqkv_d = dp("bqkv", [NL, 3 * D], f32, isOutput=False)
    bproj_d = dp("bproj", [NL, D], f32, isOutput=False)
    bfc1_d = dp("bfc1", [NL, 4 * D], f32, isOutput=False)
    bfc2_d = dp("bfc2", [NL, D], f32, isOutput=False)
    mlp1w_d = dp("mlp1w", [L, 160], f32, isOutput=False)
    mlp2w_d = dp("mlp2w", [160, 4], f32, isOutput=False)
    mlp1b_d = dp("mlp1b", [160], f32, isOutput=False)
    mlp2b_d = dp("mlp2b", [4], f32, isOutput=False)
    s_d = dp("s_bf", [T, SPC], bf16, isOutput=False)
    st_d = dp("st_f", [SPC, T], f32, isOutput=False)
    identb_d = dp("identb", [128, 128], bf16, isOutput=False)
    normg_d = dp("normg", [128, D], f32, isOutput=False)
    normb_d = dp("normb", [128, D], f32, isOutput=False)

    out_d = dp("out", [T, D], f32, isOutput=True)
    cos_d = dp("cos", [SPC, 4], f32, isOutput=True)
    pro_d = dp("pro", [SPC, 4], f32, isOutput=True)

    schunks = []
    for s in range(SPC):
        schunks += [(s, 0, 128), (s, 128, 128), (s, 256, 64)]

    with tile.TileContext(nc) as tc, ExitStack() as ctx:
        const = ctx.enter_context(tc.tile_pool(name="const", bufs=1))
        resid = ctx.enter_context(tc.tile_pool(name="resid", bufs=1))
        wpool = ctx.enter_context(tc.tile_pool(name="wpool", bufs=1))
        wqpool = ctx.enter_context(tc.tile_pool(name="wqpool", bufs=1))
        tmidp = ctx.enter_context(tc.tile_pool(name="tmidp", bufs=20))
        tfinp = ctx.enter_context(tc.tile_pool(name="tfinp", bufs=2))
        upool = ctx.enter_context(tc.tile_pool(name="upool", bufs=8))
        tcpool = ctx.enter_context(tc.tile_pool(name="tcpool", bufs=1))
        opool = ctx.enter_context(tc.tile_pool(name="opool", bufs=1))
        oupool = ctx.enter_context(tc.tile_pool(name="oupool", bufs=6))
        qkpool = ctx.enter_context(tc.tile_pool(name="qkpool", bufs=7))
        vpool = ctx.enter_context(tc.tile_pool(name="vpool", bufs=5))
        epool = ctx.enter_context(tc.tile_pool(name="epool", bufs=2))
        gpool = ctx.enter_context(tc.tile_pool(name="gpool", bufs=1))
        zpool = ctx.enter_context(tc.tile_pool(name="zpool", bufs=1))
        smallp = ctx.enter_context(tc.tile_pool(name="smallp", bufs=1))
        prodp = ctx.enter_context(tc.tile_pool(name="prodp", bufs=3))
        psA = ctx.enter_context(tc.tile_pool(name="psA", bufs=4, space="PSUM"))
        psO = ctx.enter_context(tc.tile_pool(name="psO", bufs=2, space="PSUM"))
        psN = ctx.enter_context(tc.tile_pool(name="psN", bufs=1, space="PSUM"))
        dscr = ctx.enter_context(tc.tile_pool(name="dscr", bufs=3, space="DRAM"))

        # ---------- constants ----------
        identb = const.tile([128, 128], bf16)
        nc.sync.dma_start(identb[:], identb_d[:])
        identf = const.tile([16, 16], f32)
        make_identity(nc, identf[:])
        eps_t = const.tile([128, 1], f32)
        nc.vector.memset(eps_t[:], LN_EPS)
        s_tiles = [const.tile([128, SPC], bf16, tag=f"s{i}", name=f"s{i}") for i in range(NTT)]
        for i in range(NTT):
            nc.sync.dma_start(s_tiles[i][:], s_d[i * 128:(i + 1) * 128, :])

        t_tiles = [resid.tile([128, D], f32, tag=f"t{i}", name=f"t{i}") for i in range(NTT)]
        acc_tiles = [resid.tile([128, D], f32, tag=f"a{i}", name=f"a{i}") for i in range(NTT)]
        masks_sb = const.tile([128, 4 * NTT], f32)
        cos_sb = const.tile([SPC, 4], f32)

        # ---------- helpers ----------
        def ln_normalize(tiles):
            stat = smallp.tile([128, 2 * NTT], f32, tag="lnstat")
            for i in range(NTT):
                bn6 = smallp.tile([128, 6], f32, tag="bn6")
                nc.vector.bn_stats(bn6[:], tiles[i][:])
                nc.vector.bn_aggr(stat[:, 2 * i:2 * i + 2], bn6[:])
            sd = smallp.tile([128, 2 * NTT], f32, tag="lnsd")
            nc.scalar.activation(sd[:], stat[:], AF.Sqrt, bias=eps_t[:])
            rstd = smallp.tile([128, 2 * NTT], f32, tag="lnrstd")
            nc.vector.reciprocal_approx_fast(rstd[:], sd[:])
            outs = []
            for i in range(NTT):
                u = upool.tile([128, D], bf16, tag="u", name="u")
                nc.vector.tensor_scalar(u[:], tiles[i][:],
                                        stat[:, 2 * i:2 * i + 1],
                                        rstd[:, 2 * i + 1:2 * i + 2],
                                        ALU.subtract, ALU.mult)
                outs.append(u)
            return outs

        def transpose20(u_tiles):
            chunks = [tcpool.tile([128, T], bf16, tag=f"tc{j}", name=f"tc{j}") for j in range(3)]
            for g in range(5):
                for j in range(3):
                    ps = psA.tile([128, 512], f32, tag="ps")
                    for k in range(4):
                        nc.tensor.matmul(
                            ps[:, k * 128:(k + 1) * 128],
                            u_tiles[4 * g + k][:, j * 128:(j + 1) * 128],
                            identb[:], start=True, stop=True)
                    nc.scalar.copy(chunks[j][:, g * 512:(g + 1) * 512], ps[:])
            return chunks

        def load_w(handle, li, rows, cols, tag, pool):
            tiles = []
            for c in range(rows // 128):
                wt = pool.tile([128, cols], bf16, tag=f"{tag}{c}", name=f"{tag}{c}")
                nc.sync.dma_start(wt[:], handle[li, c * 128:(c + 1) * 128, :])
                tiles.append(wt)
            return tiles

        def load_bias_col(handle, li, cols, tag):
            if zero_bias:
                return None
            n = cols // 128
            bt = smallp.tile([128, n], f32, tag=f"bc{tag}", name=f"bc{tag}")
            nc.sync.dma_start(bt[:], AP(handle, li * cols, [[1, 128], [128, n]]))
            return bt

        def load_bias_rep(handle, li, cols, tag):
            if zero_bias:
                return None
            bt = smallp.tile([128, cols], f32, tag=f"br{tag}", name=f"br{tag}")
            nc.sync.dma_start(bt[:], AP(handle, li * cols, [[0, 128], [1, cols]]))
            return bt

        # ---------- transformer block ----------
        def block(in_tiles, li, fin_writer, mid_tiles_out):
            wq = load_w(wqkv_d, li, D, 3 * D, "wq", wqpool)
            wp = load_w(wproj_d, li, D, D, "wp", wpool)
            w1 = load_w(wfc1_d, li, D, 4 * D, "w1", wpool)
            w2 = load_w(wfc2_d, li, 4 * D, D, "w2", wpool)
            bq_c = load_bias_col(bqkv_d, li, 3 * D, "qkv")
            b1_c = load_bias_col(bfc1_d, li, 4 * D, "fc1")
            bq_r = load_bias_rep(bqkv_d, li, 3 * D, "qkv")
            bp_r = load_bias_rep(bproj_d, li, D, "proj")
            b2_r = load_bias_rep(bfc2_d, li, D, "fc2")

            u_tiles = ln_normalize(in_tiles)
            uT = transpose20(u_tiles)

            # attention per sample (qk, v, scores, softmax, o)
            o_chunks = [opool.tile([128, T], bf16, tag=f"oc{c}", name=f"oc{c}") for c in range(3)]
            for s in range(SPC):
                c0s = s * L
                qk = {}
                for fc in range(6):
                    ps = psA.tile([128, 512], f32, tag="ps")
                    for kc in range(3):
                        nc.tensor.matmul(ps[:, :L],
                                         wq[kc][:, fc * 128:(fc + 1) * 128],
                                         uT[kc][:, c0s:c0s + L],
                                         start=(kc == 0), stop=(kc == 2))
                    qs = qkpool.tile([128, L], bf16, tag="qk", name="qk")
                    if zero_bias:
                        nc.scalar.copy(qs[:], ps[:, :L])
                    else:
                        nc.scalar.activation(qs[:], ps[:, :L], AF.Identity,
                                             bias=bq_c[:, fc:fc + 1])
                    qk[(s, fc)] = qs
                vt_map = {}
                for (off, size) in ((0, 128), (128, 128), (256, 64)):
                    c0 = s * L + off
                    ps = psA.tile([128, 512], f32, tag="ps")
                    for kc in range(3):
                        nc.tensor.matmul(ps[:size, :D], uT[kc][:, c0:c0 + size],
                                         wq[kc][:, 2 * D:3 * D],
                                         start=(kc == 0), stop=(kc == 2))
                    if not zero_bias:
                        nc.vector.tensor_tensor(ps[:size, :D], ps[:size, :D],
                                                bq_r[:size, 2 * D:3 * D],
                                                ALU.add)
                    vt = vpool.tile([128, H * 65], bf16, tag="v", name="v")
                    pstep = vt.ap[0][0]
                    dst = AP(vt.tensor, vt.offset,
                             [[pstep, size], [65, H], [1, DH]])
                    nc.vector.tensor_copy(dst, ps[:size, :D])
                    ones_ap = AP(vt.tensor, vt.offset + DH,
                                 [[pstep, size], [65, H], [1, 1]])
                    nc.vector.memset(ones_ap, 1.0)
                    vt_map[(s, off)] = vt
                ou_list = []
                z8 = zpool.tile([H, L], f32, tag="z8")
                for h in range(H):
                    po = (h % 2) * DH
                    q_ap = qk[(s, h // 2)][po:po + DH, :]
                    k_ap = qk[(s, 3 + h // 2)][po:po + DH, :]
                    e_ts = [(epool.tile([128, L], bf16, tag="e0", name="e0"), 128, 0),
                            (epool.tile([128, L], bf16, tag="e1", name="e1"), 128, 128),
                            (epool.tile([64, L], bf16, tag="e2", name="e2"), 64, 256)]
                    for (et, size, ko) in e_ts:
                        ps = psA.tile([128, 512], f32, tag="ps")
                        nc.tensor.matmul(ps[:size, :L], k_ap[:, ko:ko + size],
                                         q_ap[:], start=True, stop=True)
                        nc.scalar.activation(et[:size, :], ps[:size, :L],
                                             AF.Exp, scale=ATT_SCALE)
                    pso = psO.tile([65, L], f32, tag="pso")
                    for ci, (et, size, ko) in enumerate(e_ts):
                        nc.tensor.matmul(pso[:],
                                         vt_map[(s, ko)][:size, h * 65:(h + 1) * 65],
                                         et[:size, :],
                                         start=(ci == 0), stop=(ci == 2))
                    ou = oupool.tile([65, L], bf16, tag="ou")
                    nc.scalar.copy(ou[:], pso[:])
                    zr = zpool.tile([1, L], f32, tag="zr")
                    nc.scalar.copy(zr[:], pso[64:65, :])
                    nc.sync.dma_start(z8[h:h + 1, :], zr[:])
                    ou_list.append(ou)
                r8 = zpool.tile([H, L], f32, tag="r8")
                nc.vector.reciprocal_approx_fast(r8[:], z8[:])
                r8d = dscr.tile([H, L], f32, tag="r8d")
                nc.sync.dma_start(r8d[:], r8[:])
                for h in range(H):
                    rbc = zpool.tile([DH, L], f32, tag="rbc")
                    nc.sync.dma_start(
                        rbc[:], AP(r8d.tensor, r8d.offset + h * L,
                                   [[0, DH], [1, L]]))
                    c, ro = divmod(h * DH, 128)
                    nc.vector.tensor_tensor(
                        o_chunks[c][ro:ro + DH, s * L:(s + 1) * L],
                        ou_list[h][:DH, :], rbc[:], ALU.mult)

            # proj + attention residual
            for i in range(NTT):
                ps = psA.tile([128, 512], f32, tag="ps")
                for kc in range(3):
                    nc.tensor.matmul(ps[:, :D],
                                     o_chunks[kc][:, i * 128:(i + 1) * 128],
                                     wp[kc][:], start=(kc == 0), stop=(kc == 2))
                if not zero_bias:
                    nc.vector.tensor_tensor(ps[:, :D], ps[:, :D], bp_r[:],
                                            ALU.add)
                nc.vector.tensor_tensor(mid_tiles_out[i][:], in_tiles[i][:],
                                        ps[:, :D], ALU.add)

            # MLP in 512-token groups
            h2 = ln_normalize(mid_tiles_out)
            h2T = transpose20(h2)
            for tg in range(5):
                c0 = tg * 512
                g_tiles = []
                for fc in range(12):
                    ps = psA.tile([128, 512], f32, tag="ps")
                    for kc in range(3):
                        nc.tensor.matmul(ps[:],
                                         w1[kc][:, fc * 128:(fc + 1) * 128],
                                         h2T[kc][:, c0:c0 + 512],
                                         start=(kc == 0), stop=(kc == 2))
                    g = gpool.tile([128, 512], bf16, tag=f"g{fc}", name=f"g{fc}")
                    if zero_bias:
                        nc.scalar.activation(g[:], ps[:], AF.Gelu)
                    else:
                        nc.scalar.activation(g[:], ps[:], AF.Gelu,
                                             bias=b1_c[:, fc:fc + 1])
                    g_tiles.append(g)
                for k in range(4):
                    i = tg * 4 + k
                    ps = psA.tile([128, 512], f32, tag="ps")
                    for fc in range(12):
                        nc.tensor.matmul(ps[:, :D],
                                         g_tiles[fc][:, k * 128:(k + 1) * 128],
                                         w2[fc][:], start=(fc == 0),
                                         stop=(fc == 11))
                    if not zero_bias:
                        nc.vector.tensor_tensor(ps[:, :D], ps[:, :D], b2_r[:],
                                                ALU.add)
                    fin_writer(i, ps[:, :D])

        # ---------- patch embed ----------
        def patch_embed():
            convw_tags = ["wp0", "wp1", "wp2", "w20", "w21", "w22"]
            convw_t = []
            for c in range(6):
                wt = wpool.tile([128, D], bf16, tag=convw_tags[c], name=f"cw{c}")
                nc.sync.dma_start(wt[:], convw_d[c * 128:(c + 1) * 128, :])
                convw_t.append(wt)
            for i in range(NTT):
                pt = tmidp.tile([128, 768], bf16, tag="tm", name="pt")
                nc.sync.dma_start(pt[:], patches_d[i * 128:(i + 1) * 128, :])
                xpt = [qkpool.tile([128, 128], bf16, tag="qk", name=f"xpt{c}") for c in range(6)]
                for c in range(6):
                    pst = psO.tile([128, L], f32, tag="pso")
                    nc.tensor.matmul(pst[:, :128], pt[:, c * 128:(c + 1) * 128],
                                     identb[:], start=True, stop=True)
                    nc.scalar.copy(xpt[c][:], pst[:, :128])
                ps = psA.tile([128, 512], f32, tag="ps")
                for c in range(6):
                    nc.tensor.matmul(ps[:, :D], xpt[c][:], convw_t[c][:],
                                     start=(c == 0), stop=(c == 5))
                post = tmidp.tile([128, D], f32, tag="tm", name="pos")
                nc.sync.dma_start(post[:], pos_d[i * 128:(i + 1) * 128, :])
                nc.vector.tensor_tensor(t_tiles[i][:], ps[:, :D], post[:],
                                        ALU.add)

        # ---------- router ----------
        def router():
            rT = smallp.tile([128, 24], f32, tag="rT")
            for s in range(SPC):
                for kc in range(3):
                    size = 64 if kc == 2 else 128
                    g0 = s * L + kc * 128
                    left, d0 = size, 0
                    while left > 0:
                        m, r0 = divmod(g0, 128)
                        n = min(128 - r0, left)
                        nc.sync.dma_start(
                            rT[d0:d0 + n, kc * 8 + s:kc * 8 + s + 1],
                            t_tiles[m][r0:r0 + n, 0:1])
                        g0 += n
                        d0 += n
                        left -= n
            w1t = [smallp.tile([128, 160], f32, tag="m1a", name="m1a"),
                   smallp.tile([128, 160], f32, tag="m1b", name="m1b"),
                   smallp.tile([64, 160], f32, tag="m1c", name="m1c")]
            nc.sync.dma_start(w1t[0][:], mlp1w_d[0:128, :])
            nc.sync.dma_start(w1t[1][:], mlp1w_d[128:256, :])
            nc.sync.dma_start(w1t[2][:], mlp1w_d[256:320, :])
            ps1 = psN.tile([SPC, 512], f32, tag="pssm")
            for kc in range(3):
                size = 64 if kc == 2 else 128
                nc.tensor.matmul(ps1[:, :160],
                                 rT[:size, kc * 8:(kc + 1) * 8],
                                 w1t[kc][:size, :],
                                 start=(kc == 0), stop=(kc == 2))
            if not zero_bias:
                b1 = smallp.tile([SPC, 160], f32, tag="rb1")
                nc.sync.dma_start(b1[:], AP(mlp1b_d, 0, [[0, SPC], [1, 160]]))
                nc.vector.tensor_tensor(ps1[:, :160], ps1[:, :160], b1[:],
                                        ALU.add)
            r1 = smallp.tile([SPC, 160], f32, tag="r1")
            nc.scalar.activation(r1[:], ps1[:, :160], AF.Relu)
            r1T = [smallp.tile([128, SPC], f32, tag="r1Ta", name="r1Ta"),
                   smallp.tile([32, SPC], f32, tag="r1Tb", name="r1Tb")]
            for c, (n0, nn) in enumerate(((0, 128), (128, 32))):
                pst = psN.tile([128, SPC], f32, tag="psst")
                nc.tensor.matmul(pst[:nn, :], r1[:, n0:n0 + nn],
                                 identf[:SPC, :SPC], start=True, stop=True)
                nc.vector.tensor_copy(r1T[c][:nn, :], pst[:nn, :])
            w2t = [smallp.tile([128, 4], f32, tag="m2a", name="m2a"),
                   smallp.tile([32, 4], f32, tag="m2b", name="m2b")]
            nc.sync.dma_start(w2t[0][:], mlp2w_d[0:128, :])
            nc.sync.dma_start(w2t[1][:], mlp2w_d[128:160, :])
            psz = psN.tile([SPC, 512], f32, tag="pssm")
            nc.tensor.matmul(psz[:, :4], r1T[0][:], w2t[0][:],
                             start=True, stop=False)
            nc.tensor.matmul(psz[:, :4], r1T[1][:32, :], w2t[1][:],
                             start=False, stop=True)
            z = smallp.tile([SPC, 4], f32, tag="z")
            if zero_bias:
                nc.vector.tensor_copy(z[:], psz[:, :4])
            else:
                b2 = smallp.tile([SPC, 4], f32, tag="rb2")
                nc.sync.dma_start(b2[:], AP(mlp2b_d, 0, [[0, SPC], [1, 4]]))
                nc.vector.tensor_tensor(z[:], psz[:, :4], b2[:], ALU.add)
            pro = smallp.tile([SPC, 4], f32, tag="pro")
            nc.scalar.activation(pro[:], z[:], AF.Sigmoid)
            nc.sync.dma_start(pro_d[:], pro[:])
            # one-hot masks (argmax over cols 0..2, first occurrence wins)
            mk = smallp.tile([SPC, 8], f32, tag="mk")
            nc.vector.tensor_tensor(mk[:, 3:4], z[:, 1:2], z[:, 0:1], ALU.is_gt)
            nc.vector.tensor_tensor(mk[:, 4:5], z[:, 0:1], z[:, 1:2], ALU.max)
            nc.vector.tensor_tensor(mk[:, 2:3], z[:, 2:3], mk[:, 4:5], ALU.is_gt)
            nc.vector.tensor_tensor(mk[:, 5:6], mk[:, 3:4], mk[:, 2:3], ALU.mult)
            nc.vector.tensor_tensor(mk[:, 1:2], mk[:, 3:4], mk[:, 5:6],
                                    ALU.subtract)
            nc.vector.tensor_tensor(mk[:, 6:7], mk[:, 1:2], mk[:, 2:3], ALU.add)
            nc.vector.tensor_scalar(mk[:, 0:1], mk[:, 6:7], -1.0, 1.0,
                                    ALU.mult, ALU.add)
            nc.vector.memset(mk[:, 7:8], 0.0)
            for i in range(NTT):
                stt = smallp.tile([SPC, 128], f32, tag="stt")
                nc.sync.dma_start(stt[:], st_d[:, i * 128:(i + 1) * 128])
                psm = psN.tile([128, SPC], f32, tag="psst")
                nc.tensor.matmul(psm[:, :4], stt[:], mk[:, 0:4],
                                 start=True, stop=True)
                nc.scalar.copy(masks_sb[:, 4 * i:4 * i + 4], psm[:, :4])

        # ---------- run ----------
        patch_embed()

        def dense_writer(i, ps_ap):
            nc.vector.tensor_tensor(t_tiles[i][:], t_tiles[i][:], ps_ap,
                                    ALU.add)

        for li in range(ndense):
            with nc.named_scope(f"layer{li}"):
                block(t_tiles, li, dense_writer, t_tiles)

        if nlate > 0:
            with nc.named_scope("router"):
                router()
            ps_ssm = psN.tile([SPC, 512], f32, tag="pssm")
            for i in range(NTT):
                pr = prodp.tile([128, D], bf16, tag="prod")
                nc.vector.tensor_tensor(pr[:], t_tiles[i][:], t_tiles[i][:],
                                        ALU.mult)
                nc.tensor.matmul(ps_ssm[:, :D], s_tiles[i][:], pr[:],
                                 start=(i == 0), stop=(i == NTT - 1))
            ssm = smallp.tile([SPC, D], f32, tag="ssm")
            nc.vector.tensor_copy(ssm[:], ps_ssm[:, :D])

            for lb in range(nlate):
                li = START + 1 + lb
                tmid = [tmidp.tile([128, D], f32, tag="tm", name="tm") for _ in range(NTT)]
                psbox = {}

                def late_writer(i, ps_ap, lb=lb, psbox=psbox, tmid=tmid):
                    if i == 0:
                        psbox["num"] = psN.tile([SPC, 512], f32, tag="pssm",
                                                name="ps_num")
                        psbox["sst"] = psN.tile([128, 512], f32, tag="psst",
                                                name="ps_sst")
                    ps_num, ps_sst = psbox["num"], psbox["sst"]
                    tf = tfinp.tile([128, D], f32, tag="tf")
                    nc.vector.tensor_tensor(tf[:], tmid[i][:], ps_ap, ALU.add)
                    pr = prodp.tile([128, D], bf16, tag="prod")
                    nc.vector.tensor_tensor(pr[:], t_tiles[i][:], tf[:],
                                            ALU.mult)
                    nc.tensor.matmul(ps_num[:, :D], s_tiles[i][:], pr[:],
                                     start=(i == 0), stop=(i == NTT - 1))
                    pr2 = prodp.tile([128, D], bf16, tag="prod2")
                    nc.vector.tensor_tensor(pr2[:], tf[:], tf[:], ALU.mult)
                    nc.tensor.matmul(ps_sst[:SPC, :D], s_tiles[i][:], pr2[:],
                                     start=(i == 0), stop=(i == NTT - 1))
                    mcol = masks_sb[:, 4 * i + lb:4 * i + lb + 1]
                    if lb == 0:
                        nc.vector.tensor_scalar_mul(acc_tiles[i][:], tf[:],
                                                    mcol)
                    else:
                        nc.vector.scalar_tensor_tensor(acc_tiles[i][:], tf[:],
                                                       mcol, acc_tiles[i][:],
                                                       ALU.mult, ALU.add)

                with nc.named_scope(f"late{lb}"):
                    block(t_tiles, li, late_writer, tmid)
                ps_num, ps_sst = psbox["num"], psbox["sst"]
                v8 = prodp.tile([SPC, D], f32, tag="v8", name="v8")
                nc.vector.tensor_tensor(v8[:], ssm[:], ps_sst[:SPC, :D],
                                        ALU.mult)
                v8b = prodp.tile([SPC, D], f32, tag="v8", name="v8")
                nc.scalar.activation(v8b[:], v8[:], AF.Sqrt)
                nc.vector.tensor_scalar_max(v8b[:], v8b[:], 1e-8)
                v8c = prodp.tile([SPC, D], f32, tag="v8", name="v8")
                nc.vector.reciprocal_approx_fast(v8c[:], v8b[:])
                v8d = prodp.tile([SPC, D], f32, tag="v8", name="v8")
                nc.vector.tensor_tensor(v8d[:], ps_num[:, :D], v8c[:],
                                        ALU.mult)
                qs = smallp.tile([SPC, 1], f32, tag="qs")
                nc.vector.tensor_reduce(qs[:], v8d[:], mybir.AxisListType.X,
                                        ALU.add)
                nc.vector.tensor_scalar_mul(cos_sb[:, lb:lb + 1], qs[:],
                                            1.0 / D)
            nc.vector.memset(cos_sb[:, 3:4], 0.0)
            nc.sync.dma_start(cos_d[:], cos_sb[:])
            final_in = acc_tiles
        else:
            final_in = t_tiles
            nc.vector.memset(cos_sb[:], 0.0)
            nc.sync.dma_start(cos_d[:], cos_sb[:])
            pro0 = smallp.tile([SPC, 4], f32, tag="pro")
            nc.vector.memset(pro0[:], 0.0)
            nc.sync.dma_start(pro_d[:], pro0[:])

        # ---------- final layernorm + output ----------
        statF = smallp.tile([128, 2 * NTT], f32, tag="lnstat")
        for i in range(NTT):
            bn6 = smallp.tile([128, 6], f32, tag="bn6")
            nc.vector.bn_stats(bn6[:], final_in[i][:])
            nc.vector.bn_aggr(statF[:, 2 * i:2 * i + 2], bn6[:])
        sdF = smallp.tile([128, 2 * NTT], f32, tag="lnsd")
        nc.scalar.activation(sdF[:], statF[:], AF.Sqrt, bias=eps_t[:])
        rstdF = smallp.tile([128, 2 * NTT], f32, tag="lnrstd")
        nc.vector.reciprocal_approx_fast(rstdF[:], sdF[:])
        if not trivial_norm:
            gt = const.tile([128, D], f32, tag="ngt")
            bt = const.tile([128, D], f32, tag="nbt")
            nc.sync.dma_start(gt[:], normg_d[:])
            nc.sync.dma_start(bt[:], normb_d[:])
        for i in range(NTT):
            o = tfinp.tile([128, D], f32, tag="tf", name="tf")
            nc.vector.tensor_scalar(o[:], final_in[i][:],
                                    statF[:, 2 * i:2 * i + 1],
                                    rstdF[:, 2 * i + 1:2 * i + 2],
                                    ALU.subtract, ALU.mult)
            if not trivial_norm:
                nc.vector.tensor_tensor(o[:], o[:], gt[:], ALU.mult)
                nc.vector.tensor_tensor(o[:], o[:], bt[:], ALU.add)
            nc.sync.dma_start(out_d[i * 128:(i + 1) * 128, :], o[:])

    nc.compile()
    return nc


def _host_prep(inputs):
    import ml_dtypes
    bf = ml_dtypes.bfloat16
    f32 = np.float32

    conv_w = np.asarray(inputs["conv_w"], f32)
    conv_b = np.asarray(inputs["conv_b"], f32)
    pos_z = np.asarray(inputs["pos_z"], f32)[0]
    pos_x = np.asarray(inputs["pos_x"], f32)[0]
    ln1_g = np.asarray(inputs["ln1_g"], f32)
    ln1_b = np.asarray(inputs["ln1_b"], f32)
    ln2_g = np.asarray(inputs["ln2_g"], f32)
    ln2_b = np.asarray(inputs["ln2_b"], f32)
    qkv_w = np.asarray(inputs["qkv_w"], f32)
    qkv_b = np.asarray(inputs["qkv_b"], f32)
    proj_b = np.asarray(inputs["proj_b"], f32)
    fc1_w = np.asarray(inputs["fc1_w"], f32)
    fc1_b = np.asarray(inputs["fc1_b"], f32)
    fc2_b = np.asarray(inputs["fc2_b"], f32)

    wqkv = (ln1_g[:, :, None] * qkv_w).astype(bf)
    bqkv = (np.einsum("ld,ldo->lo", ln1_b, qkv_w) + qkv_b).astype(f32)
    wfc1 = (ln2_g[:, :, None] * fc1_w).astype(bf)
    bfc1 = (np.einsum("ld,ldo->lo", ln2_b, fc1_w) + fc1_b).astype(f32)

    zero_bias = not (np.any(bqkv) or np.any(proj_b) or np.any(bfc1)
                     or np.any(fc2_b) or np.any(np.asarray(inputs["mlp1_b"]))
                     or np.any(np.asarray(inputs["mlp2_b"])))

    pos_sample = np.concatenate([pos_z, pos_x], axis=0) + conv_b[None, :]
    pos_full = np.tile(pos_sample, (SPC, 1)).astype(f32)

    convw = np.ascontiguousarray(conv_w.reshape(D, 768).T).astype(bf)

    S = np.zeros((T, SPC), np.float32)
    S[np.arange(T), np.arange(T) // L] = 1.0

    mlp2w = np.zeros((160, 4), f32)
    mlp2w[:, :3] = np.asarray(inputs["mlp2_w"], f32)
    mlp2b = np.zeros((4,), f32)
    mlp2b[:3] = np.asarray(inputs["mlp2_b"], f32)

    norm_g = np.asarray(inputs["norm_g"], f32)
    norm_b = np.asarray(inputs["norm_b"], f32)
    trivial_norm = bool(np.all(norm_g == 1.0) and np.all(norm_b == 0.0))

    common = {
        "convw": convw,
        "pos": pos_full,
        "wqkv": np.ascontiguousarray(wqkv),
        "wproj": np.asarray(inputs["proj_w"], f32).astype(bf),
        "wfc1": np.ascontiguousarray(wfc1),
        "wfc2": np.asarray(inputs["fc2_w"], f32).astype(bf),
        "bqkv": bqkv, "bproj": proj_b, "bfc1": bfc1, "bfc2": fc2_b,
        "mlp1w": np.asarray(inputs["mlp1_w"], f32),
        "mlp2w": mlp2w,
        "mlp1b": np.asarray(inputs["mlp1_b"], f32),
        "mlp2b": mlp2b,
        "s_bf": S.astype(bf),
        "st_f": np.ascontiguousarray(S.T),
        "identb": np.eye(128, dtype=bf),
        "normg": np.tile(norm_g[None, :], (128, 1)).astype(f32),
        "normb": np.tile(norm_b[None, :], (128, 1)).astype(f32),
    }
    z = np.asarray(inputs["z"], f32)
    x = np.asarray(inputs["x"], f32)
    # im2col: patch features ordered (c, dy, dx) to match convw layout
    zp = z.reshape(B, 3, 8, 16, 8, 16).transpose(0, 2, 4, 1, 3, 5)
    zp = zp.reshape(B, 64, 768)
    xp = x.reshape(B, 3, 16, 16, 16, 16).transpose(0, 2, 4, 1, 3, 5)
    xp = xp.reshape(B, 256, 768)
    patches = np.concatenate([zp, xp], axis=1).astype(bf)  # [B, 320, 768]
    in_maps = []
    for c in range(NCORES):
        m = dict(common)
        m["patches"] = np.ascontiguousarray(
            patches[c * SPC:(c + 1) * SPC].reshape(T, 768))
        in_maps.append(m)
    return in_maps, zero_bias, trivial_norm


def kernel(**inputs):
    import os
    from concourse.bass_utils import run_bass_kernel_spmd

    ndense = int(os.environ.get("KERNEL_NDENSE", START + 1))
    nlate = int(os.environ.get("KERNEL_NLATE", NL - START - 1))

    in_maps, zero_bias, trivial_norm = _host_prep(inputs)
    key = (ndense, nlate, zero_bias, trivial_norm)
    if key not in _CACHE:
        _CACHE[key] = _build(*key)
    nc = _CACHE[key]

    trace = bool(int(os.environ.get("KERNEL_TRACE", "0")))
    res = run_bass_kernel_spmd(nc, in_maps, list(range(NCORES)), trace=trace)
    global LAST_EXEC_NS, LAST_SCOPES
    LAST_EXEC_NS = res.exec_time_ns
    LAST_SCOPES = res.per_core_scope_times
    outs, coss, pros = [], [], []
    for c in range(NCORES):
        r = res.results[c]
        outs.append(np.asarray(r["out"], np.float32).reshape(SPC, L, D))
        coss.append(np.asarray(r["cos"], np.float32)[:, :3])
        pros.append(np.asarray(r["pro"], np.float32)[:, :3])
    return (np.concatenate(outs, axis=0), np.concatenate(coss, axis=0),
            np.concatenate(pros, axis=0))


# revision 16
# speedup vs baseline: 1.0026x; 1.0026x over previous
"""Trainium2 Bass kernel for nn_BaseBackbone_78194174591299 (ViT + top-1 routing).

Sharding: data-parallel over batch — 8 samples per core x 8 NeuronCores.

Key identity: in the reference's masked dispatch, rows with sel==i still hold
mid when block i runs, so block(t,i)[sel==i] == block(mid,i)[sel==i]. Each
late block (9..11) is computed once on mid, serving both the dispatch (via a
one-hot blend) and the cosine stats.

Device program per core (token-major fp32 residual, bf16 matmul operands):
  patch-embed (im2col DMA + transpose-via-identity-matmul + matmul) -> t
  9 dense blocks (in-place residual)
  router MLP (fp32) -> logits z -> one-hot masks (exact 0/1) + sigmoid pro
  3 late blocks on mid -> blend into acc + cosine stats
  final layernorm -> out

Self-contained: all shapes hardcoded; nothing read from /root/problem.
"""
import numpy as np

B = 64
NCORES = 8
SPC = B // NCORES       # 8 samples per core
L = 320
D = 384
H = 6
DH = D // H             # 64
NL = 12
START = 8
T = SPC * L             # 2560
NTT = T // 128          # 20
LN_EPS = 1e-6
ATT_SCALE = DH ** -0.5

_CACHE = {}
LAST_EXEC_NS = None
LAST_SCOPES = None
LAST_INSTS = None


def _build(ndense, nlate, zero_bias, trivial_norm):
    from contextlib import ExitStack
    import concourse.bass as bass
    import concourse.tile as tile
    from concourse import bacc, mybir
    from concourse.masks import make_identity

    f32 = mybir.dt.float32
    bf16 = mybir.dt.bfloat16
    AF = mybir.ActivationFunctionType
    ALU = mybir.AluOpType
    AP = bass.AP

    nc = bacc.Bacc("TRN2", target_bir_lowering=False, debug=False)
    dp = nc.declare_dram_parameter

    patches_d = dp("patches", [T, 768], bf16, isOutput=False)
    convw_d = dp("convw", [768, D], bf16, isOutput=False)
    pos_d = dp("pos", [T, D], f32, isOutput=False)
    wqkv_d = dp("wqkv", [NL, D, 3 * D], bf16, isOutput=False)
    wproj_d = dp("wproj", [NL, D, D], bf16, isOutput=False)
    wfc1_d = dp("wfc1", [NL, D, 4 * D], bf16, isOutput=False)
    wfc2_d = dp("wfc2", [NL, 4 * D, D], bf16, isOutput=False)
    bqkv_d = dp("bqkv", [NL, 3 * D], f32, isOutput=False)
    bproj_d = dp("bproj", [NL, D], f32, isOutput=False)
    bfc1_d = dp("bfc1", [NL, 4 * D], f32, isOutput=False)
    bfc2_d = dp("bfc2", [NL, D], f32, isOutput=False)
    mlp1w_d = dp("mlp1w", [L, 160], f32, isOutput=False)
    mlp2w_d = dp("mlp2w", [160, 4], f32, isOutput=False)
    mlp1b_d = dp("mlp1b", [160], f32, isOutput=False)
    mlp2b_d = dp("mlp2b", [4], f32, isOutput=False)
    s_d = dp("s_bf", [T, SPC], bf16, isOutput=False)
    st_d = dp("st_f", [SPC, T], f32, isOutput=False)
    identb_d = dp("identb", [128, 128], bf16, isOutput=False)
    normg_d = dp("normg", [128, D], f32, isOutput=False)
    normb_d = dp("normb", [128, D], f32, isOutput=False)

    out_d = dp("out", [T, D], f32, isOutput=True)
    cos_d = dp("cos", [SPC, 4], f32, isOutput=True)
    pro_d = dp("pro", [SPC, 4], f32, isOutput=True)

    schunks = []
    for s in range(SPC):
        schunks += [(s, 0, 128), (s, 128, 128), (s, 256, 64)]

    with tile.TileContext(nc) as tc, ExitStack() as ctx:
        const = ctx.enter_context(tc.tile_pool(name="const", bufs=1))
        resid = ctx.enter_context(tc.tile_pool(name="resid", bufs=1))
        wpool = ctx.enter_context(tc.tile_pool(name="wpool", bufs=1))
        wqpool = ctx.enter_context(tc.tile_pool(name="wqpool", bufs=1))
        tmidp = ctx.enter_context(tc.tile_pool(name="tmidp", bufs=20))
        tfinp = ctx.enter_context(tc.tile_pool(name="tfinp", bufs=2))
        upool = ctx.enter_context(tc.tile_pool(name="upool", bufs=8))
        tcpool = ctx.enter_context(tc.tile_pool(name="tcpool", bufs=1))
        opool = ctx.enter_context(tc.tile_pool(name="opool", bufs=1))
        oupool = ctx.enter_context(tc.tile_pool(name="oupool", bufs=6))
        qkpool = ctx.enter_context(tc.tile_pool(name="qkpool", bufs=7))
        vpool = ctx.enter_context(tc.tile_pool(name="vpool", bufs=5))
        epool = ctx.enter_context(tc.tile_pool(name="epool", bufs=2))
        gpool = ctx.enter_context(tc.tile_pool(name="gpool", bufs=1))
        zpool = ctx.enter_context(tc.tile_pool(name="zpool", bufs=1))
        smallp = ctx.enter_context(tc.tile_pool(name="smallp", bufs=1))
        prodp = ctx.enter_context(tc.tile_pool(name="prodp", bufs=3))
        psA = ctx.enter_context(tc.tile_pool(name="psA", bufs=4, space="PSUM"))
        psO = ctx.enter_context(tc.tile_pool(name="psO", bufs=2, space="PSUM"))
        psN = ctx.enter_context(tc.tile_pool(name="psN", bufs=1, space="PSUM"))
        dscr = ctx.enter_context(tc.tile_pool(name="dscr", bufs=3, space="DRAM"))

        # ---------- constants ----------
        identb = const.tile([128, 128], bf16)
        nc.sync.dma_start(identb[:], identb_d[:])
        identf = const.tile([16, 16], f32)
        make_identity(nc, identf[:])
        eps_t = const.tile([128, 1], f32)
        nc.vector.memset(eps_t[:], LN_EPS)
        s_tiles = [const.tile([128, SPC], bf16, tag=f"s{i}", name=f"s{i}") for i in range(NTT)]
        for i in range(NTT):
            nc.sync.dma_start(s_tiles[i][:], s_d[i * 128:(i + 1) * 128, :])

        t_tiles = [resid.tile([128, D], f32, tag=f"t{i}", name=f"t{i}") for i in range(NTT)]
        acc_tiles = [resid.tile([128, D], f32, tag=f"a{i}", name=f"a{i}") for i in range(NTT)]
        masks_sb = const.tile([128, 4 * NTT], f32)
        cos_sb = const.tile([SPC, 4], f32)

        # ---------- helpers ----------
        def ln_normalize(tiles):
            stat = smallp.tile([128, 2 * NTT], f32, tag="lnstat")
            for i in range(NTT):
                bn6 = smallp.tile([128, 6], f32, tag="bn6")
                nc.vector.bn_stats(bn6[:], tiles[i][:])
                nc.vector.bn_aggr(stat[:, 2 * i:2 * i + 2], bn6[:])
            sd = smallp.tile([128, 2 * NTT], f32, tag="lnsd")
            nc.scalar.activation(sd[:], stat[:], AF.Sqrt, bias=eps_t[:])
            rstd = smallp.tile([128, 2 * NTT], f32, tag="lnrstd")
            nc.vector.reciprocal_approx_fast(rstd[:], sd[:])
            outs = []
            for i in range(NTT):
                u = upool.tile([128, D], bf16, tag="u", name="u")
                nc.vector.tensor_scalar(u[:], tiles[i][:],
                                        stat[:, 2 * i:2 * i + 1],
                                        rstd[:, 2 * i + 1:2 * i + 2],
                                        ALU.subtract, ALU.mult)
                outs.append(u)
            return outs

        def transpose20(u_tiles):
            chunks = [tcpool.tile([128, T], bf16, tag=f"tc{j}", name=f"tc{j}") for j in range(3)]
            for g in range(5):
                for j in range(3):
                    ps = psA.tile([128, 512], f32, tag="ps")
                    for k in range(4):
                        nc.tensor.matmul(
                            ps[:, k * 128:(k + 1) * 128],
                            u_tiles[4 * g + k][:, j * 128:(j + 1) * 128],
                            identb[:], start=True, stop=True)
                    nc.scalar.copy(chunks[j][:, g * 512:(g + 1) * 512], ps[:])
            return chunks

        def load_w(handle, li, rows, cols, tag, pool):
            tiles = []
            for c in range(rows // 128):
                wt = pool.tile([128, cols], bf16, tag=f"{tag}{c}", name=f"{tag}{c}")
                nc.sync.dma_start(wt[:], handle[li, c * 128:(c + 1) * 128, :])
                tiles.append(wt)
            return tiles

        def load_bias_col(handle, li, cols, tag):
            if zero_bias:
                return None
            n = cols // 128
            bt = smallp.tile([128, n], f32, tag=f"bc{tag}", name=f"bc{tag}")
            nc.sync.dma_start(bt[:], AP(handle, li * cols, [[1, 128], [128, n]]))
            return bt

        def load_bias_rep(handle, li, cols, tag):
            if zero_bias:
                return None
            bt = smallp.tile([128, cols], f32, tag=f"br{tag}", name=f"br{tag}")
            nc.sync.dma_start(bt[:], AP(handle, li * cols, [[0, 128], [1, cols]]))
            return bt

        # ---------- transformer block ----------
        def block(in_tiles, li, fin_writer, mid_tiles_out):
            wq = load_w(wqkv_d, li, D, 3 * D, "wq", wqpool)
            wp = load_w(wproj_d, li, D, D, "wp", wpool)
            w1 = load_w(wfc1_d, li, D, 4 * D, "w1", wpool)
            w2 = load_w(wfc2_d, li, 4 * D, D, "w2", wpool)
            bq_c = load_bias_col(bqkv_d, li, 3 * D, "qkv")
            b1_c = load_bias_col(bfc1_d, li, 4 * D, "fc1")
            bq_r = load_bias_rep(bqkv_d, li, 3 * D, "qkv")
            bp_r = load_bias_rep(bproj_d, li, D, "proj")
            b2_r = load_bias_rep(bfc2_d, li, D, "fc2")

            u_tiles = ln_normalize(in_tiles)
            uT = transpose20(u_tiles)

            # attention per sample (qk, v, scores, softmax, o)
            o_chunks = [opool.tile([128, T], bf16, tag=f"oc{c}", name=f"oc{c}") for c in range(3)]
            for s in range(SPC):
                c0s = s * L
                qk = {}
                for fc in range(6):
                    ps = psA.tile([128, 512], f32, tag="ps")
                    for kc in range(3):
                        nc.tensor.matmul(ps[:, :L],
                                         wq[kc][:, fc * 128:(fc + 1) * 128],
                                         uT[kc][:, c0s:c0s + L],
                                         start=(kc == 0), stop=(kc == 2))
                    qs = qkpool.tile([128, L], bf16, tag="qk", name="qk")
                    if zero_bias:
                        nc.scalar.copy(qs[:], ps[:, :L])
                    else:
                        nc.scalar.activation(qs[:], ps[:, :L], AF.Identity,
                                             bias=bq_c[:, fc:fc + 1])
                    qk[(s, fc)] = qs
                vt_map = {}
                for (off, size) in ((0, 128), (128, 128), (256, 64)):
                    c0 = s * L + off
                    ps = psA.tile([128, 512], f32, tag="ps")
                    for kc in range(3):
                        nc.tensor.matmul(ps[:size, :D], uT[kc][:, c0:c0 + size],
                                         wq[kc][:, 2 * D:3 * D],
                                         start=(kc == 0), stop=(kc == 2))
                    if not zero_bias:
                        nc.vector.tensor_tensor(ps[:size, :D], ps[:size, :D],
                                                bq_r[:size, 2 * D:3 * D],
                                                ALU.add)
                    vt = vpool.tile([128, H * 65], bf16, tag="v", name="v")
                    pstep = vt.ap[0][0]
                    dst = AP(vt.tensor, vt.offset,
                             [[pstep, size], [65, H], [1, DH]])
                    nc.vector.tensor_copy(dst, ps[:size, :D])
                    ones_ap = AP(vt.tensor, vt.offset + DH,
                                 [[pstep, size], [65, H], [1, 1]])
                    nc.vector.memset(ones_ap, 1.0)
                    vt_map[(s, off)] = vt
                ou_list = []
                z8 = zpool.tile([H, L], f32, tag="z8")
                for h in range(H):
                    po = (h % 2) * DH
                    q_ap = qk[(s, h // 2)][po:po + DH, :]
                    k_ap = qk[(s, 3 + h // 2)][po:po + DH, :]
                    e_ts = [(epool.tile([128, L], bf16, tag="e0", name="e0"), 128, 0),
                            (epool.tile([128, L], bf16, tag="e1", name="e1"), 128, 128),
                            (epool.tile([64, L], bf16, tag="e2", name="e2"), 64, 256)]
                    for (et, size, ko) in e_ts:
                        ps = psA.tile([128, 512], f32, tag="ps")
                        nc.tensor.matmul(ps[:size, :L], k_ap[:, ko:ko + size],
                                         q_ap[:], start=True, stop=True)
                        nc.scalar.activation(et[:size, :], ps[:size, :L],
                                             AF.Exp, scale=ATT_SCALE)
                    pso = psO.tile([65, L], f32, tag="pso")
                    for ci, (et, size, ko) in enumerate(e_ts):
                        nc.tensor.matmul(pso[:],
                                         vt_map[(s, ko)][:size, h * 65:(h + 1) * 65],
                                         et[:size, :],
                                         start=(ci == 0), stop=(ci == 2))
                    ou = oupool.tile([65, L], bf16, tag="ou")
                    nc.scalar.copy(ou[:], pso[:])
                    zr = zpool.tile([1, L], f32, tag="zr")
                    nc.scalar.copy(zr[:], pso[64:65, :])
                    nc.sync.dma_start(z8[h:h + 1, :], zr[:])
                    ou_list.append(ou)
                r8 = zpool.tile([H, L], f32, tag="r8")
                nc.vector.reciprocal_approx_fast(r8[:], z8[:])
                r8d = dscr.tile([H, L], f32, tag="r8d")
                nc.sync.dma_start(r8d[:], r8[:])
                for h in range(H):
                    rbc = zpool.tile([DH, L], f32, tag="rbc")
                    nc.sync.dma_start(
                        rbc[:], AP(r8d.tensor, r8d.offset + h * L,
                                   [[0, DH], [1, L]]))
                    c, ro = divmod(h * DH, 128)
                    nc.vector.tensor_tensor(
                        o_chunks[c][ro:ro + DH, s * L:(s + 1) * L],
                        ou_list[h][:DH, :], rbc[:], ALU.mult)

            # proj + attention residual
            for i in range(NTT):
                ps = psA.tile([128, 512], f32, tag="ps")
                for kc in range(3):
                    nc.tensor.matmul(ps[:, :D],
                                     o_chunks[kc][:, i * 128:(i + 1) * 128],
                                     wp[kc][:], start=(kc == 0), stop=(kc == 2))
                if not zero_bias:
                    nc.vector.tensor_tensor(ps[:, :D], ps[:, :D], bp_r[:],
                                            ALU.add)
                nc.vector.tensor_tensor(mid_tiles_out[i][:], in_tiles[i][:],
                                        ps[:, :D], ALU.add)

            # MLP in 512-token groups
            h2 = ln_normalize(mid_tiles_out)
            h2T = transpose20(h2)
            for tg in range(5):
                c0 = tg * 512
                g_tiles = []
                for fc in range(12):
                    ps = psA.tile([128, 512], f32, tag="ps")
                    for kc in range(3):
                        nc.tensor.matmul(ps[:],
                                         w1[kc][:, fc * 128:(fc + 1) * 128],
                                         h2T[kc][:, c0:c0 + 512],
                                         start=(kc == 0), stop=(kc == 2))
                    g = gpool.tile([128, 512], bf16, tag=f"g{fc}", name=f"g{fc}")
                    if zero_bias:
                        nc.scalar.activation(g[:], ps[:], AF.Gelu)
                    else:
                        nc.scalar.activation(g[:], ps[:], AF.Gelu,
                                             bias=b1_c[:, fc:fc + 1])
                    g_tiles.append(g)
                for k in range(4):
                    i = tg * 4 + k
                    ps = psA.tile([128, 512], f32, tag="ps")
                    for fc in range(12):
                        nc.tensor.matmul(ps[:, :D],
                                         g_tiles[fc][:, k * 128:(k + 1) * 128],
                                         w2[fc][:], start=(fc == 0),
                                         stop=(fc == 11))
                    if not zero_bias:
                        nc.vector.tensor_tensor(ps[:, :D], ps[:, :D], b2_r[:],
                                                ALU.add)
                    fin_writer(i, ps[:, :D])

        # ---------- patch embed ----------
        def patch_embed():
            convw_tags = ["wp0", "wp1", "wp2", "w20", "w21", "w22"]
            convw_t = []
            for c in range(6):
                wt = wpool.tile([128, D], bf16, tag=convw_tags[c], name=f"cw{c}")
                nc.sync.dma_start(wt[:], convw_d[c * 128:(c + 1) * 128, :])
                convw_t.append(wt)
            for i in range(NTT):
                pt = tmidp.tile([128, 768], bf16, tag="tm", name="pt")
                nc.sync.dma_start(pt[:], patches_d[i * 128:(i + 1) * 128, :])
                xpt = [qkpool.tile([128, 128], bf16, tag="qk", name=f"xpt{c}") for c in range(6)]
                for c in range(6):
                    pst = psO.tile([128, L], f32, tag="pso")
                    nc.tensor.matmul(pst[:, :128], pt[:, c * 128:(c + 1) * 128],
                                     identb[:], start=True, stop=True)
                    nc.scalar.copy(xpt[c][:], pst[:, :128])
                ps = psA.tile([128, 512], f32, tag="ps")
                for c in range(6):
                    nc.tensor.matmul(ps[:, :D], xpt[c][:], convw_t[c][:],
                                     start=(c == 0), stop=(c == 5))
                post = tmidp.tile([128, D], f32, tag="tm", name="pos")
                nc.sync.dma_start(post[:], pos_d[i * 128:(i + 1) * 128, :])
                nc.vector.tensor_tensor(t_tiles[i][:], ps[:, :D], post[:],
                                        ALU.add)

        # ---------- router ----------
        def router():
            rT = smallp.tile([128, 24], f32, tag="rT")
            for s in range(SPC):
                for kc in range(3):
                    size = 64 if kc == 2 else 128
                    g0 = s * L + kc * 128
                    left, d0 = size, 0
                    while left > 0:
                        m, r0 = divmod(g0, 128)
                        n = min(128 - r0, left)
                        nc.sync.dma_start(
                            rT[d0:d0 + n, kc * 8 + s:kc * 8 + s + 1],
                            t_tiles[m][r0:r0 + n, 0:1])
                        g0 += n
                        d0 += n
                        left -= n
            w1t = [smallp.tile([128, 160], f32, tag="m1a", name="m1a"),
                   smallp.tile([128, 160], f32, tag="m1b", name="m1b"),
                   smallp.tile([64, 160], f32, tag="m1c", name="m1c")]
            nc.sync.dma_start(w1t[0][:], mlp1w_d[0:128, :])
            nc.sync.dma_start(w1t[1][:], mlp1w_d[128:256, :])
            nc.sync.dma_start(w1t[2][:], mlp1w_d[256:320, :])
            ps1 = psN.tile([SPC, 512], f32, tag="pssm")
            for kc in range(3):
                size = 64 if kc == 2 else 128
                nc.tensor.matmul(ps1[:, :160],
                                 rT[:size, kc * 8:(kc + 1) * 8],
                                 w1t[kc][:size, :],
                                 start=(kc == 0), stop=(kc == 2))
            if not zero_bias:
                b1 = smallp.tile([SPC, 160], f32, tag="rb1")
                nc.sync.dma_start(b1[:], AP(mlp1b_d, 0, [[0, SPC], [1, 160]]))
                nc.vector.tensor_tensor(ps1[:, :160], ps1[:, :160], b1[:],
                                        ALU.add)
            r1 = smallp.tile([SPC, 160], f32, tag="r1")
            nc.scalar.activation(r1[:], ps1[:, :160], AF.Relu)
            r1T = [smallp.tile([128, SPC], f32, tag="r1Ta", name="r1Ta"),
                   smallp.tile([32, SPC], f32, tag="r1Tb", name="r1Tb")]
            for c, (n0, nn) in enumerate(((0, 128), (128, 32))):
                pst = psN.tile([128, SPC], f32, tag="psst")
                nc.tensor.matmul(pst[:nn, :], r1[:, n0:n0 + nn],
                                 identf[:SPC, :SPC], start=True, stop=True)
                nc.vector.tensor_copy(r1T[c][:nn, :], pst[:nn, :])
            w2t = [smallp.tile([128, 4], f32, tag="m2a", name="m2a"),
                   smallp.tile([32, 4], f32, tag="m2b", name="m2b")]
            nc.sync.dma_start(w2t[0][:], mlp2w_d[0:128, :])
            nc.sync.dma_start(w2t[1][:], mlp2w_d[128:160, :])
            psz = psN.tile([SPC, 512], f32, tag="pssm")
            nc.tensor.matmul(psz[:, :4], r1T[0][:], w2t[0][:],
                             start=True, stop=False)
            nc.tensor.matmul(psz[:, :4], r1T[1][:32, :], w2t[1][:],
                             start=False, stop=True)
            z = smallp.tile([SPC, 4], f32, tag="z")
            if zero_bias:
                nc.vector.tensor_copy(z[:], psz[:, :4])
            else:
                b2 = smallp.tile([SPC, 4], f32, tag="rb2")
                nc.sync.dma_start(b2[:], AP(mlp2b_d, 0, [[0, SPC], [1, 4]]))
                nc.vector.tensor_tensor(z[:], psz[:, :4], b2[:], ALU.add)
            pro = smallp.tile([SPC, 4], f32, tag="pro")
            nc.scalar.activation(pro[:], z[:], AF.Sigmoid)
            nc.sync.dma_start(pro_d[:], pro[:])
            # one-hot masks (argmax over cols 0..2, first occurrence wins)
            mk = smallp.tile([SPC, 8], f32, tag="mk")
            nc.vector.tensor_tensor(mk[:, 3:4], z[:, 1:2], z[:, 0:1], ALU.is_gt)
            nc.vector.tensor_tensor(mk[:, 4:5], z[:, 0:1], z[:, 1:2], ALU.max)
            nc.vector.tensor_tensor(mk[:, 2:3], z[:, 2:3], mk[:, 4:5], ALU.is_gt)
            nc.vector.tensor_tensor(mk[:, 5:6], mk[:, 3:4], mk[:, 2:3], ALU.mult)
            nc.vector.tensor_tensor(mk[:, 1:2], mk[:, 3:4], mk[:, 5:6],
                                    ALU.subtract)
            nc.vector.tensor_tensor(mk[:, 6:7], mk[:, 1:2], mk[:, 2:3], ALU.add)
            nc.vector.tensor_scalar(mk[:, 0:1], mk[:, 6:7], -1.0, 1.0,
                                    ALU.mult, ALU.add)
            nc.vector.memset(mk[:, 7:8], 0.0)
            for i in range(NTT):
                stt = smallp.tile([SPC, 128], f32, tag="stt")
                nc.sync.dma_start(stt[:], st_d[:, i * 128:(i + 1) * 128])
                psm = psN.tile([128, SPC], f32, tag="psst")
                nc.tensor.matmul(psm[:, :4], stt[:], mk[:, 0:4],
                                 start=True, stop=True)
                nc.scalar.copy(masks_sb[:, 4 * i:4 * i + 4], psm[:, :4])

        # ---------- run ----------
        patch_embed()

        def dense_writer(i, ps_ap):
            nc.vector.tensor_tensor(t_tiles[i][:], t_tiles[i][:], ps_ap,
                                    ALU.add)

        for li in range(ndense):
            with nc.named_scope(f"layer{li}"):
                block(t_tiles, li, dense_writer, t_tiles)

        if nlate > 0:
            with nc.named_scope("router"):
                router()
            ps_ssm = psN.tile([SPC, 512], f32, tag="pssm")
            for i in range(NTT):
                pr = prodp.tile([128, D], bf16, tag="prod")
                nc.vector.tensor_tensor(pr[:], t_tiles[i][:], t_tiles[i][:],
                                        ALU.mult)
                nc.tensor.matmul(ps_ssm[:, :D], s_tiles[i][:], pr[:],
                                 start=(i == 0), stop=(i == NTT - 1))
            ssm = smallp.tile([SPC, D], f32, tag="ssm")
            nc.vector.tensor_copy(ssm[:], ps_ssm[:, :D])

            for lb in range(nlate):
                li = START + 1 + lb
                tmid = [tmidp.tile([128, D], f32, tag="tm", name="tm") for _ in range(NTT)]
                psbox = {}

                def late_writer(i, ps_ap, lb=lb, psbox=psbox, tmid=tmid):
                    if i == 0:
                        psbox["num"] = psN.tile([SPC, 512], f32, tag="pssm",
                                                name="ps_num")
                        psbox["sst"] = psN.tile([128, 512], f32, tag="psst",
                                                name="ps_sst")
                    ps_num, ps_sst = psbox["num"], psbox["sst"]
                    tf = tfinp.tile([128, D], f32, tag="tf")
                    nc.vector.tensor_tensor(tf[:], tmid[i][:], ps_ap, ALU.add)
                    pr = prodp.tile([128, D], bf16, tag="prod")
                    nc.vector.tensor_tensor(pr[:], t_tiles[i][:], tf[:],
                                            ALU.mult)
                    nc.tensor.matmul(ps_num[:, :D], s_tiles[i][:], pr[:],
                                     start=(i == 0), stop=(i == NTT - 1))
                    pr2 = prodp.tile([128, D], bf16, tag="prod2")
                    nc.vector.tensor_tensor(pr2[:], tf[:], tf[:], ALU.mult)
                    nc.tensor.matmul(ps_sst[:SPC, :D], s_tiles[i][:], pr2[:],
                                     start=(i == 0), stop=(i == NTT - 1))
                    mcol = masks_sb[:, 4 * i + lb:4 * i + lb + 1]
                    if lb == 0:
                        nc.vector.tensor_scalar_mul(acc_tiles[i][:], tf[:],
                                                    mcol)
                    else:
                        nc.vector.scalar_tensor_tensor(acc_tiles[i][:], tf[:],
                                                       mcol, acc_tiles[i][:],
                                                       ALU.mult, ALU.add)

                with nc.named_scope(f"late{lb}"):
                    block(t_tiles, li, late_writer, tmid)
                ps_num, ps_sst = psbox["num"], psbox["sst"]
                v8 = prodp.tile([SPC, D], f32, tag="v8", name="v8")
                nc.vector.tensor_tensor(v8[:], ssm[:], ps_sst[:SPC, :D],
                                        ALU.mult)
                v8b = prodp.tile([SPC, D], f32, tag="v8", name="v8")
                nc.scalar.activation(v8b[:], v8[:], AF.Sqrt)
                nc.vector.tensor_scalar_max(v8b[:], v8b[:], 1e-8)
                v8c = prodp.tile([SPC, D], f32, tag="v8", name="v8")
                nc.vector.reciprocal_approx_fast(v8c[:], v8b[:])
                v8d = prodp.tile([SPC, D], f32, tag="v8", name="v8")
                nc.vector.tensor_tensor(v8d[:], ps_num[:, :D], v8c[:],
                                        ALU.mult)
                qs = smallp.tile([SPC, 1], f32, tag="qs")
                nc.vector.tensor_reduce(qs[:], v8d[:], mybir.AxisListType.X,
                                        ALU.add)
                nc.vector.tensor_scalar_mul(cos_sb[:, lb:lb + 1], qs[:],
                                            1.0 / D)
            nc.vector.memset(cos_sb[:, 3:4], 0.0)
            nc.sync.dma_start(cos_d[:], cos_sb[:])
            final_in = acc_tiles
        else:
            final_in = t_tiles
            nc.vector.memset(cos_sb[:], 0.0)
            nc.sync.dma_start(cos_d[:], cos_sb[:])
            pro0 = smallp.tile([SPC, 4], f32, tag="pro")
            nc.vector.memset(pro0[:], 0.0)
            nc.sync.dma_start(pro_d[:], pro0[:])

        # ---------- final layernorm + output ----------
        statF = smallp.tile([128, 2 * NTT], f32, tag="lnstat")
        for i in range(NTT):
            bn6 = smallp.tile([128, 6], f32, tag="bn6")
            nc.vector.bn_stats(bn6[:], final_in[i][:])
            nc.vector.bn_aggr(statF[:, 2 * i:2 * i + 2], bn6[:])
        sdF = smallp.tile([128, 2 * NTT], f32, tag="lnsd")
        nc.scalar.activation(sdF[:], statF[:], AF.Sqrt, bias=eps_t[:])
        rstdF = smallp.tile([128, 2 * NTT], f32, tag="lnrstd")
        nc.vector.reciprocal_approx_fast(rstdF[:], sdF[:])
        if not trivial_norm:
            gt = const.tile([128, D], f32, tag="ngt")
            bt = const.tile([128, D], f32, tag="nbt")
            nc.sync.dma_start(gt[:], normg_d[:])
            nc.sync.dma_start(bt[:], normb_d[:])
        for i in range(NTT):
            o = tfinp.tile([128, D], f32, tag="tf", name="tf")
            nc.vector.tensor_scalar(o[:], final_in[i][:],
                                    statF[:, 2 * i:2 * i + 1],
                                    rstdF[:, 2 * i + 1:2 * i + 2],
                                    ALU.subtract, ALU.mult)
            if not trivial_norm:
                nc.vector.tensor_tensor(o[:], o[:], gt[:], ALU.mult)
                nc.vector.tensor_tensor(o[:], o[:], bt[:], ALU.add)
            nc.sync.dma_start(out_d[i * 128:(i + 1) * 128, :], o[:])

    nc.compile()
    return nc


def _host_prep(inputs):
    import ml_dtypes
    bf = ml_dtypes.bfloat16
    f32 = np.float32

    conv_w = np.asarray(inputs["conv_w"], f32)
    conv_b = np.asarray(inputs["conv_b"], f32)
    pos_z = np.asarray(inputs["pos_z"], f32)[0]
    pos_x = np.asarray(inputs["pos_x"], f32)[0]
    ln1_g = np.asarray(inputs["ln1_g"], f32)
    ln1_b = np.asarray(inputs["ln1_b"], f32)
    ln2_g = np.asarray(inputs["ln2_g"], f32)
    ln2_b = np.asarray(inputs["ln2_b"], f32)
    qkv_w = np.asarray(inputs["qkv_w"], f32)
    qkv_b = np.asarray(inputs["qkv_b"], f32)
    proj_b = np.asarray(inputs["proj_b"], f32)
    fc1_w = np.asarray(inputs["fc1_w"], f32)
    fc1_b = np.asarray(inputs["fc1_b"], f32)
    fc2_b = np.asarray(inputs["fc2_b"], f32)

    wqkv = (ln1_g[:, :, None] * qkv_w).astype(bf)
    bqkv = (np.einsum("ld,ldo->lo", ln1_b, qkv_w) + qkv_b).astype(f32)
    wfc1 = (ln2_g[:, :, None] * fc1_w).astype(bf)
    bfc1 = (np.einsum("ld,ldo->lo", ln2_b, fc1_w) + fc1_b).astype(f32)

    zero_bias = not (np.any(bqkv) or np.any(proj_b) or np.any(bfc1)
                     or np.any(fc2_b) or np.any(np.asarray(inputs["mlp1_b"]))
                     or np.any(np.asarray(inputs["mlp2_b"])))

    pos_sample = np.concatenate([pos_z, pos_x], axis=0) + conv_b[None, :]
    pos_full = np.tile(pos_sample, (SPC, 1)).astype(f32)

    convw = np.ascontiguousarray(conv_w.reshape(D, 768).T).astype(bf)

    S = np.zeros((T, SPC), np.float32)
    S[np.arange(T), np.arange(T) // L] = 1.0

    mlp2w = np.zeros((160, 4), f32)
    mlp2w[:, :3] = np.asarray(inputs["mlp2_w"], f32)
    mlp2b = np.zeros((4,), f32)
    mlp2b[:3] = np.asarray(inputs["mlp2_b"], f32)

    norm_g = np.asarray(inputs["norm_g"], f32)
    norm_b = np.asarray(inputs["norm_b"], f32)
    trivial_norm = bool(np.all(norm_g == 1.0) and np.all(norm_b == 0.0))

    common = {
        "convw": convw,
        "pos": pos_full,
        "wqkv": np.ascontiguousarray(wqkv),
        "wproj": np.asarray(inputs["proj_w"], f32).astype(bf),
        "wfc1": np.ascontiguousarray(wfc1),
        "wfc2": np.asarray(inputs["fc2_w"], f32).astype(bf),
        "bqkv": bqkv, "bproj": proj_b, "bfc1": bfc1, "bfc2": fc2_b,
        "mlp1w": np.asarray(inputs["mlp1_w"], f32),
        "mlp2w": mlp2w,
        "mlp1b": np.asarray(inputs["mlp1_b"], f32),
        "mlp2b": mlp2b,
        "s_bf": S.astype(bf),
        "st_f": np.ascontiguousarray(S.T),
        "identb": np.eye(128, dtype=bf),
        "normg": np.tile(norm_g[None, :], (128, 1)).astype(f32),
        "normb": np.tile(norm_b[None, :], (128, 1)).astype(f32),
    }
    z = np.asarray(inputs["z"], f32)
    x = np.asarray(inputs["x"], f32)
    # im2col: patch features ordered (c, dy, dx) to match convw layout
    zp = z.reshape(B, 3, 8, 16, 8, 16).transpose(0, 2, 4, 1, 3, 5)
    zp = zp.reshape(B, 64, 768)
    xp = x.reshape(B, 3, 16, 16, 16, 16).transpose(0, 2, 4, 1, 3, 5)
    xp = xp.reshape(B, 256, 768)
    patches = np.concatenate([zp, xp], axis=1).astype(bf)  # [B, 320, 768]
    in_maps = []
    for c in range(NCORES):
        m = dict(common)
        m["patches"] = np.ascontiguousarray(
            patches[c * SPC:(c + 1) * SPC].reshape(T, 768))
        in_maps.append(m)
    return in_maps, zero_bias, trivial_norm


def kernel(**inputs):
    import os
    from concourse.bass_utils import run_bass_kernel_spmd

    ndense = int(os.environ.get("KERNEL_NDENSE", START + 1))
    nlate = int(os.environ.get("KERNEL_NLATE", NL - START - 1))

    in_maps, zero_bias, trivial_norm = _host_prep(inputs)
    key = (ndense, nlate, zero_bias, trivial_norm)
    if key not in _CACHE:
        _CACHE[key] = _build(*key)
    nc = _CACHE[key]

    trace = bool(int(os.environ.get("KERNEL_TRACE", "0")))
    res = run_bass_kernel_spmd(nc, in_maps, list(range(NCORES)), trace=trace)
    global LAST_EXEC_NS, LAST_SCOPES, LAST_INSTS
    LAST_EXEC_NS = res.exec_time_ns
    LAST_SCOPES = res.per_core_scope_times
    LAST_INSTS = res.instructions_and_trace
    outs, coss, pros = [], [], []
    for c in range(NCORES):
        r = res.results[c]
        outs.append(np.asarray(r["out"], np.float32).reshape(SPC, L, D))
        coss.append(np.asarray(r["cos"], np.float32)[:, :3])
        pros.append(np.asarray(r["pro"], np.float32)[:, :3])
    return (np.concatenate(outs, axis=0), np.concatenate(coss, axis=0),
            np.concatenate(pros, axis=0))


# revision 18
# speedup vs baseline: 1.0204x; 1.0177x over previous
"""Trainium2 Bass kernel for nn_BaseBackbone_78194174591299 (ViT + top-1 routing).

Sharding: data-parallel over batch — 8 samples per core x 8 NeuronCores.

Key identity: in the reference's masked dispatch, rows with sel==i still hold
mid when block i runs, so block(t,i)[sel==i] == block(mid,i)[sel==i]. Each
late block (9..11) is computed once on mid, serving both the dispatch (via a
one-hot blend) and the cosine stats.

Device program per core (token-major fp32 residual, bf16 matmul operands):
  patch-embed (im2col DMA + transpose-via-identity-matmul + matmul) -> t
  9 dense blocks (in-place residual)
  router MLP (fp32) -> logits z -> one-hot masks (exact 0/1) + sigmoid pro
  3 late blocks on mid -> blend into acc + cosine stats
  final layernorm -> out

Self-contained: all shapes hardcoded; nothing read from /root/problem.
"""
import numpy as np

B = 64
NCORES = 8
SPC = B // NCORES       # 8 samples per core
L = 320
D = 384
H = 6
DH = D // H             # 64
NL = 12
START = 8
T = SPC * L             # 2560
NTT = T // 128          # 20
LN_EPS = 1e-6
ATT_SCALE = DH ** -0.5

_CACHE = {}
LAST_EXEC_NS = None
LAST_SCOPES = None
LAST_INSTS = None


def _build(ndense, nlate, zero_bias, trivial_norm):
    from contextlib import ExitStack
    import concourse.bass as bass
    import concourse.tile as tile
    from concourse import bacc, mybir
    from concourse.masks import make_identity

    f32 = mybir.dt.float32
    bf16 = mybir.dt.bfloat16
    AF = mybir.ActivationFunctionType
    ALU = mybir.AluOpType
    AP = bass.AP

    nc = bacc.Bacc("TRN2", target_bir_lowering=False, debug=False)
    dp = nc.declare_dram_parameter

    patches_d = dp("patches", [T, 768], bf16, isOutput=False)
    convw_d = dp("convw", [768, D], bf16, isOutput=False)
    pos_d = dp("pos", [T, D], f32, isOutput=False)
    wqkv_d = dp("wqkv", [NL, D, 3 * D], bf16, isOutput=False)
    wproj_d = dp("wproj", [NL, D, D], bf16, isOutput=False)
    wfc1_d = dp("wfc1", [NL, D, 4 * D], bf16, isOutput=False)
    wfc2_d = dp("wfc2", [NL, 4 * D, D], bf16, isOutput=False)
    bqkv_d = dp("bqkv", [NL, 3 * D], f32, isOutput=False)
    bproj_d = dp("bproj", [NL, D], f32, isOutput=False)
    bfc1_d = dp("bfc1", [NL, 4 * D], f32, isOutput=False)
    bfc2_d = dp("bfc2", [NL, D], f32, isOutput=False)
    mlp1w_d = dp("mlp1w", [L, 160], f32, isOutput=False)
    mlp2w_d = dp("mlp2w", [160, 4], f32, isOutput=False)
    mlp1b_d = dp("mlp1b", [160], f32, isOutput=False)
    mlp2b_d = dp("mlp2b", [4], f32, isOutput=False)
    s_d = dp("s_bf", [T, SPC], bf16, isOutput=False)
    st_d = dp("st_f", [SPC, T], f32, isOutput=False)
    identb_d = dp("identb", [128, 128], bf16, isOutput=False)
    normg_d = dp("normg", [128, D], f32, isOutput=False)
    normb_d = dp("normb", [128, D], f32, isOutput=False)

    out_d = dp("out", [T, D], f32, isOutput=True)
    cos_d = dp("cos", [SPC, 4], f32, isOutput=True)
    pro_d = dp("pro", [SPC, 4], f32, isOutput=True)

    schunks = []
    for s in range(SPC):
        schunks += [(s, 0, 128), (s, 128, 128), (s, 256, 64)]

    with tile.TileContext(nc) as tc, ExitStack() as ctx:
        const = ctx.enter_context(tc.tile_pool(name="const", bufs=1))
        resid = ctx.enter_context(tc.tile_pool(name="resid", bufs=1))
        wpool = ctx.enter_context(tc.tile_pool(name="wpool", bufs=1))
        wqpool = ctx.enter_context(tc.tile_pool(name="wqpool", bufs=1))
        tmidp = ctx.enter_context(tc.tile_pool(name="tmidp", bufs=20))
        tfinp = ctx.enter_context(tc.tile_pool(name="tfinp", bufs=2))
        upool = ctx.enter_context(tc.tile_pool(name="upool", bufs=8))
        tcpool = ctx.enter_context(tc.tile_pool(name="tcpool", bufs=1))
        opool = ctx.enter_context(tc.tile_pool(name="opool", bufs=1))
        oupool = ctx.enter_context(tc.tile_pool(name="oupool", bufs=6))
        qkpool = ctx.enter_context(tc.tile_pool(name="qkpool", bufs=7))
        vpool = ctx.enter_context(tc.tile_pool(name="vpool", bufs=5))
        epool = ctx.enter_context(tc.tile_pool(name="epool", bufs=2))
        gpool = ctx.enter_context(tc.tile_pool(name="gpool", bufs=1))
        zpool = ctx.enter_context(tc.tile_pool(name="zpool", bufs=1))
        smallp = ctx.enter_context(tc.tile_pool(name="smallp", bufs=1))
        prodp = ctx.enter_context(tc.tile_pool(name="prodp", bufs=3))
        psA = ctx.enter_context(tc.tile_pool(name="psA", bufs=4, space="PSUM"))
        psO = ctx.enter_context(tc.tile_pool(name="psO", bufs=2, space="PSUM"))
        psN = ctx.enter_context(tc.tile_pool(name="psN", bufs=1, space="PSUM"))
        dscr = ctx.enter_context(tc.tile_pool(name="dscr", bufs=3, space="DRAM"))

        # ---------- constants ----------
        identb = const.tile([128, 128], bf16)
        nc.sync.dma_start(identb[:], identb_d[:])
        identf = const.tile([16, 16], f32)
        make_identity(nc, identf[:])
        eps_t = const.tile([128, 1], f32)
        nc.vector.memset(eps_t[:], LN_EPS)
        s_tiles = [const.tile([128, SPC], bf16, tag=f"s{i}", name=f"s{i}") for i in range(NTT)]
        for i in range(NTT):
            nc.sync.dma_start(s_tiles[i][:], s_d[i * 128:(i + 1) * 128, :])

        t_tiles = [resid.tile([128, D], f32, tag=f"t{i}", name=f"t{i}") for i in range(NTT)]
        acc_tiles = [resid.tile([128, D], f32, tag=f"a{i}", name=f"a{i}") for i in range(NTT)]
        masks_sb = const.tile([128, 4 * NTT], f32)
        cos_sb = const.tile([SPC, 4], f32)

        # ---------- helpers ----------
        def new_stat(name):
            return smallp.tile([128, 2 * NTT], f32, tag=name, name=name)

        def collect(stat, i, ap):
            bn6 = smallp.tile([128, 6], f32, tag="bn6")
            nc.vector.bn_stats(bn6[:], ap)
            nc.vector.bn_aggr(stat[:, 2 * i:2 * i + 2], bn6[:])

        def stats_pass(tiles, name):
            stat = new_stat(name)
            for i in range(NTT):
                collect(stat, i, tiles[i][:])
            return stat

        def finish_ln(stat):
            sd = smallp.tile([128, 2 * NTT], f32, tag="lnsd")
            nc.scalar.activation(sd[:], stat[:], AF.Sqrt, bias=eps_t[:])
            rstd = smallp.tile([128, 2 * NTT], f32, tag="lnrstd")
            nc.vector.reciprocal_approx_fast(rstd[:], sd[:])
            return rstd

        def ln_normalize(tiles, stat):
            rstd = finish_ln(stat)
            outs = []
            for i in range(NTT):
                u = upool.tile([128, D], bf16, tag="u", name="u")
                nc.vector.tensor_scalar(u[:], tiles[i][:],
                                        stat[:, 2 * i:2 * i + 1],
                                        rstd[:, 2 * i + 1:2 * i + 2],
                                        ALU.subtract, ALU.mult)
                outs.append(u)
            return outs

        def transpose20(u_tiles):
            chunks = [tcpool.tile([128, T], bf16, tag=f"tc{j}", name=f"tc{j}") for j in range(3)]
            for g in range(5):
                for j in range(3):
                    ps = psA.tile([128, 512], f32, tag="ps")
                    for k in range(4):
                        nc.tensor.matmul(
                            ps[:, k * 128:(k + 1) * 128],
                            u_tiles[4 * g + k][:, j * 128:(j + 1) * 128],
                            identb[:], start=True, stop=True)
                    nc.scalar.copy(chunks[j][:, g * 512:(g + 1) * 512], ps[:])
            return chunks

        def load_w(handle, li, rows, cols, tag, pool):
            tiles = []
            for c in range(rows // 128):
                wt = pool.tile([128, cols], bf16, tag=f"{tag}{c}", name=f"{tag}{c}")
                nc.sync.dma_start(wt[:], handle[li, c * 128:(c + 1) * 128, :])
                tiles.append(wt)
            return tiles

        def load_bias_col(handle, li, cols, tag):
            if zero_bias:
                return None
            n = cols // 128
            bt = smallp.tile([128, n], f32, tag=f"bc{tag}", name=f"bc{tag}")
            nc.sync.dma_start(bt[:], AP(handle, li * cols, [[1, 128], [128, n]]))
            return bt

        def load_bias_rep(handle, li, cols, tag):
            if zero_bias:
                return None
            bt = smallp.tile([128, cols], f32, tag=f"br{tag}", name=f"br{tag}")
            nc.sync.dma_start(bt[:], AP(handle, li * cols, [[0, 128], [1, cols]]))
            return bt

        # ---------- transformer block ----------
        def block(in_tiles, li, fin_writer, mid_tiles_out, ln1_stat):
            wq = load_w(wqkv_d, li, D, 3 * D, "wq", wqpool)
            wp = load_w(wproj_d, li, D, D, "wp", wpool)
            w1 = load_w(wfc1_d, li, D, 4 * D, "w1", wpool)
            w2 = load_w(wfc2_d, li, 4 * D, D, "w2", wpool)
            bq_c = load_bias_col(bqkv_d, li, 3 * D, "qkv")
            b1_c = load_bias_col(bfc1_d, li, 4 * D, "fc1")
            bq_r = load_bias_rep(bqkv_d, li, 3 * D, "qkv")
            bp_r = load_bias_rep(bproj_d, li, D, "proj")
            b2_r = load_bias_rep(bfc2_d, li, D, "fc2")

            stat2 = new_stat("ln2s")
            u_tiles = ln_normalize(in_tiles, ln1_stat)
            uT = transpose20(u_tiles)

            # attention per sample (qk, v, scores, softmax, o)
            o_chunks = [opool.tile([128, T], bf16, tag=f"oc{c}", name=f"oc{c}") for c in range(3)]
            for s in range(SPC):
                c0s = s * L
                qk = {}
                for fc in range(6):
                    ps = psA.tile([128, 512], f32, tag="ps")
                    for kc in range(3):
                        nc.tensor.matmul(ps[:, :L],
                                         wq[kc][:, fc * 128:(fc + 1) * 128],
                                         uT[kc][:, c0s:c0s + L],
                                         start=(kc == 0), stop=(kc == 2))
                    qs = qkpool.tile([128, L], bf16, tag="qk", name="qk")
                    if zero_bias:
                        nc.vector.tensor_copy(qs[:], ps[:, :L])
                    else:
                        nc.scalar.activation(qs[:], ps[:, :L], AF.Identity,
                                             bias=bq_c[:, fc:fc + 1])
                    qk[(s, fc)] = qs
                vt_map = {}
                for (off, size) in ((0, 128), (128, 128), (256, 64)):
                    c0 = s * L + off
                    ps = psA.tile([128, 512], f32, tag="ps")
                    for kc in range(3):
                        nc.tensor.matmul(ps[:size, :D], uT[kc][:, c0:c0 + size],
                                         wq[kc][:, 2 * D:3 * D],
                                         start=(kc == 0), stop=(kc == 2))
                    if not zero_bias:
                        nc.vector.tensor_tensor(ps[:size, :D], ps[:size, :D],
                                                bq_r[:size, 2 * D:3 * D],
                                                ALU.add)
                    vt = vpool.tile([128, H * 65], bf16, tag="v", name="v")
                    pstep = vt.ap[0][0]
                    dst = AP(vt.tensor, vt.offset,
                             [[pstep, size], [65, H], [1, DH]])
                    nc.vector.tensor_copy(dst, ps[:size, :D])
                    ones_ap = AP(vt.tensor, vt.offset + DH,
                                 [[pstep, size], [65, H], [1, 1]])
                    nc.vector.memset(ones_ap, 1.0)
                    vt_map[(s, off)] = vt
                ou_list = []
                z8 = zpool.tile([H, L], f32, tag="z8")
                for h in range(H):
                    po = (h % 2) * DH
                    q_ap = qk[(s, h // 2)][po:po + DH, :]
                    k_ap = qk[(s, 3 + h // 2)][po:po + DH, :]
                    e_ts = [(epool.tile([128, L], bf16, tag="e0", name="e0"), 128, 0),
                            (epool.tile([128, L], bf16, tag="e1", name="e1"), 128, 128),
                            (epool.tile([64, L], bf16, tag="e2", name="e2"), 64, 256)]
                    for (et, size, ko) in e_ts:
                        ps = psA.tile([128, 512], f32, tag="ps")
                        nc.tensor.matmul(ps[:size, :L], k_ap[:, ko:ko + size],
                                         q_ap[:], start=True, stop=True,
                                         tile_position=(po, 0))
                        nc.scalar.activation(et[:size, :], ps[:size, :L],
                                             AF.Exp, scale=ATT_SCALE)
                    pso = psO.tile([65, L], f32, tag="pso")
                    for ci, (et, size, ko) in enumerate(e_ts):
                        nc.tensor.matmul(pso[:],
                                         vt_map[(s, ko)][:size, h * 65:(h + 1) * 65],
                                         et[:size, :],
                                         start=(ci == 0), stop=(ci == 2))
                    ou = oupool.tile([65, L], bf16, tag="ou")
                    nc.vector.tensor_copy(ou[:], pso[:])
                    zr = zpool.tile([1, L], f32, tag="zr")
                    nc.scalar.copy(zr[:], pso[64:65, :])
                    nc.sync.dma_start(z8[h:h + 1, :], zr[:])
                    ou_list.append(ou)
                r8 = zpool.tile([H, L], f32, tag="r8")
                nc.vector.reciprocal_approx_fast(r8[:], z8[:])
                r8d = dscr.tile([H, L], f32, tag="r8d")
                nc.sync.dma_start(r8d[:], r8[:])
                for h in range(H):
                    rbc = zpool.tile([DH, L], f32, tag="rbc")
                    nc.sync.dma_start(
                        rbc[:], AP(r8d.tensor, r8d.offset + h * L,
                                   [[0, DH], [1, L]]))
                    c, ro = divmod(h * DH, 128)
                    nc.vector.tensor_tensor(
                        o_chunks[c][ro:ro + DH, s * L:(s + 1) * L],
                        ou_list[h][:DH, :], rbc[:], ALU.mult)

            # proj + attention residual
            for i in range(NTT):
                ps = psA.tile([128, 512], f32, tag="ps")
                for kc in range(3):
                    nc.tensor.matmul(ps[:, :D],
                                     o_chunks[kc][:, i * 128:(i + 1) * 128],
                                     wp[kc][:], start=(kc == 0), stop=(kc == 2))
                if not zero_bias:
                    nc.vector.tensor_tensor(ps[:, :D], ps[:, :D], bp_r[:],
                                            ALU.add)
                nc.vector.tensor_tensor(mid_tiles_out[i][:], in_tiles[i][:],
                                        ps[:, :D], ALU.add)
                collect(stat2, i, mid_tiles_out[i][:])

            # MLP in 512-token groups
            h2 = ln_normalize(mid_tiles_out, stat2)
            h2T = transpose20(h2)
            for tg in range(5):
                c0 = tg * 512
                g_tiles = []
                for fc in range(12):
                    ps = psA.tile([128, 512], f32, tag="ps")
                    for kc in range(3):
                        nc.tensor.matmul(ps[:],
                                         w1[kc][:, fc * 128:(fc + 1) * 128],
                                         h2T[kc][:, c0:c0 + 512],
                                         start=(kc == 0), stop=(kc == 2))
                    g = gpool.tile([128, 512], bf16, tag=f"g{fc}", name=f"g{fc}")
                    if zero_bias:
                        nc.scalar.activation(g[:], ps[:], AF.Gelu)
                    else:
                        nc.scalar.activation(g[:], ps[:], AF.Gelu,
                                             bias=b1_c[:, fc:fc + 1])
                    g_tiles.append(g)
                for k in range(4):
                    i = tg * 4 + k
                    ps = psA.tile([128, 512], f32, tag="ps")
                    for fc in range(12):
                        nc.tensor.matmul(ps[:, :D],
                                         g_tiles[fc][:, k * 128:(k + 1) * 128],
                                         w2[fc][:], start=(fc == 0),
                                         stop=(fc == 11))
                    if not zero_bias:
                        nc.vector.tensor_tensor(ps[:, :D], ps[:, :D], b2_r[:],
                                                ALU.add)
                    fin_writer(i, ps[:, :D])

        # ---------- patch embed ----------
        def patch_embed():
            convw_tags = ["wp0", "wp1", "wp2", "w20", "w21", "w22"]
            convw_t = []
            for c in range(6):
                wt = wpool.tile([128, D], bf16, tag=convw_tags[c], name=f"cw{c}")
                nc.sync.dma_start(wt[:], convw_d[c * 128:(c + 1) * 128, :])
                convw_t.append(wt)
            for i in range(NTT):
                pt = tmidp.tile([128, 768], bf16, tag="tm", name="pt")
                nc.sync.dma_start(pt[:], patches_d[i * 128:(i + 1) * 128, :])
                xpt = [qkpool.tile([128, 128], bf16, tag="qk", name=f"xpt{c}") for c in range(6)]
                for c in range(6):
                    pst = psO.tile([128, L], f32, tag="pso")
                    nc.tensor.matmul(pst[:, :128], pt[:, c * 128:(c + 1) * 128],
                                     identb[:], start=True, stop=True)
                    nc.scalar.copy(xpt[c][:], pst[:, :128])
                ps = psA.tile([128, 512], f32, tag="ps")
                for c in range(6):
                    nc.tensor.matmul(ps[:, :D], xpt[c][:], convw_t[c][:],
                                     start=(c == 0), stop=(c == 5))
                post = tmidp.tile([128, D], f32, tag="tm", name="pos")
                nc.sync.dma_start(post[:], pos_d[i * 128:(i + 1) * 128, :])
                nc.vector.tensor_tensor(t_tiles[i][:], ps[:, :D], post[:],
                                        ALU.add)
                collect(pe_stat, i, t_tiles[i][:])

        # ---------- router ----------
        def router():
            rT = smallp.tile([128, 24], f32, tag="rT")
            for s in range(SPC):
                for kc in range(3):
                    size = 64 if kc == 2 else 128
                    g0 = s * L + kc * 128
                    left, d0 = size, 0
                    while left > 0:
                        m, r0 = divmod(g0, 128)
                        n = min(128 - r0, left)
                        nc.sync.dma_start(
                            rT[d0:d0 + n, kc * 8 + s:kc * 8 + s + 1],
                            t_tiles[m][r0:r0 + n, 0:1])
                        g0 += n
                        d0 += n
                        left -= n
            w1t = [smallp.tile([128, 160], f32, tag="m1a", name="m1a"),
                   smallp.tile([128, 160], f32, tag="m1b", name="m1b"),
                   smallp.tile([64, 160], f32, tag="m1c", name="m1c")]
            nc.sync.dma_start(w1t[0][:], mlp1w_d[0:128, :])
            nc.sync.dma_start(w1t[1][:], mlp1w_d[128:256, :])
            nc.sync.dma_start(w1t[2][:], mlp1w_d[256:320, :])
            ps1 = psN.tile([SPC, 512], f32, tag="pssm")
            for kc in range(3):
                size = 64 if kc == 2 else 128
                nc.tensor.matmul(ps1[:, :160],
                                 rT[:size, kc * 8:(kc + 1) * 8],
                                 w1t[kc][:size, :],
                                 start=(kc == 0), stop=(kc == 2))
            if not zero_bias:
                b1 = smallp.tile([SPC, 160], f32, tag="rb1")
                nc.sync.dma_start(b1[:], AP(mlp1b_d, 0, [[0, SPC], [1, 160]]))
                nc.vector.tensor_tensor(ps1[:, :160], ps1[:, :160], b1[:],
                                        ALU.add)
            r1 = smallp.tile([SPC, 160], f32, tag="r1")
            nc.scalar.activation(r1[:], ps1[:, :160], AF.Relu)
            r1T = [smallp.tile([128, SPC], f32, tag="r1Ta", name="r1Ta"),
                   smallp.tile([32, SPC], f32, tag="r1Tb", name="r1Tb")]
            for c, (n0, nn) in enumerate(((0, 128), (128, 32))):
                pst = psN.tile([128, SPC], f32, tag="psst")
                nc.tensor.matmul(pst[:nn, :], r1[:, n0:n0 + nn],
                                 identf[:SPC, :SPC], start=True, stop=True)
                nc.vector.tensor_copy(r1T[c][:nn, :], pst[:nn, :])
            w2t = [smallp.tile([128, 4], f32, tag="m2a", name="m2a"),
                   smallp.tile([32, 4], f32, tag="m2b", name="m2b")]
            nc.sync.dma_start(w2t[0][:], mlp2w_d[0:128, :])
            nc.sync.dma_start(w2t[1][:], mlp2w_d[128:160, :])
            psz = psN.tile([SPC, 512], f32, tag="pssm")
            nc.tensor.matmul(psz[:, :4], r1T[0][:], w2t[0][:],
                             start=True, stop=False)
            nc.tensor.matmul(psz[:, :4], r1T[1][:32, :], w2t[1][:],
                             start=False, stop=True)
            z = smallp.tile([SPC, 4], f32, tag="z")
            if zero_bias:
                nc.vector.tensor_copy(z[:], psz[:, :4])
            else:
                b2 = smallp.tile([SPC, 4], f32, tag="rb2")
                nc.sync.dma_start(b2[:], AP(mlp2b_d, 0, [[0, SPC], [1, 4]]))
                nc.vector.tensor_tensor(z[:], psz[:, :4], b2[:], ALU.add)
            pro = smallp.tile([SPC, 4], f32, tag="pro")
            nc.scalar.activation(pro[:], z[:], AF.Sigmoid)
            nc.sync.dma_start(pro_d[:], pro[:])
            # one-hot masks (argmax over cols 0..2, first occurrence wins)
            mk = smallp.tile([SPC, 8], f32, tag="mk")
            nc.vector.tensor_tensor(mk[:, 3:4], z[:, 1:2], z[:, 0:1], ALU.is_gt)
            nc.vector.tensor_tensor(mk[:, 4:5], z[:, 0:1], z[:, 1:2], ALU.max)
            nc.vector.tensor_tensor(mk[:, 2:3], z[:, 2:3], mk[:, 4:5], ALU.is_gt)
            nc.vector.tensor_tensor(mk[:, 5:6], mk[:, 3:4], mk[:, 2:3], ALU.mult)
            nc.vector.tensor_tensor(mk[:, 1:2], mk[:, 3:4], mk[:, 5:6],
                                    ALU.subtract)
            nc.vector.tensor_tensor(mk[:, 6:7], mk[:, 1:2], mk[:, 2:3], ALU.add)
            nc.vector.tensor_scalar(mk[:, 0:1], mk[:, 6:7], -1.0, 1.0,
                                    ALU.mult, ALU.add)
            nc.vector.memset(mk[:, 7:8], 0.0)
            for i in range(NTT):
                stt = smallp.tile([SPC, 128], f32, tag="stt")
                nc.sync.dma_start(stt[:], st_d[:, i * 128:(i + 1) * 128])
                psm = psN.tile([128, SPC], f32, tag="psst")
                nc.tensor.matmul(psm[:, :4], stt[:], mk[:, 0:4],
                                 start=True, stop=True)
                nc.scalar.copy(masks_sb[:, 4 * i:4 * i + 4], psm[:, :4])

        # ---------- run ----------
        pe_stat = new_stat("st1")
        patch_embed()

        carry = {"stat": pe_stat}

        for li in range(ndense):
            nstat = new_stat(f"st{li % 2}")

            def dense_writer(i, ps_ap, nstat=nstat):
                nc.vector.tensor_tensor(t_tiles[i][:], t_tiles[i][:], ps_ap,
                                        ALU.add)
                collect(nstat, i, t_tiles[i][:])

            with nc.named_scope(f"layer{li}"):
                block(t_tiles, li, dense_writer, t_tiles, carry["stat"])
            carry["stat"] = nstat

        if nlate > 0:
            with nc.named_scope("router"):
                router()
            ps_ssm = psN.tile([SPC, 512], f32, tag="pssm")
            for i in range(NTT):
                pr = prodp.tile([128, D], bf16, tag="prod")
                nc.vector.tensor_tensor(pr[:], t_tiles[i][:], t_tiles[i][:],
                                        ALU.mult)
                nc.tensor.matmul(ps_ssm[:, :D], s_tiles[i][:], pr[:],
                                 start=(i == 0), stop=(i == NTT - 1))
            ssm = smallp.tile([SPC, D], f32, tag="ssm")
            nc.vector.tensor_copy(ssm[:], ps_ssm[:, :D])

            fstat_box = {"s": new_stat("stF")}
            for lb in range(nlate):
                li = START + 1 + lb
                tmid = [tmidp.tile([128, D], f32, tag="tm", name="tm") for _ in range(NTT)]
                psbox = {}

                def late_writer(i, ps_ap, lb=lb, psbox=psbox, tmid=tmid):
                    if i == 0:
                        psbox["num"] = psN.tile([SPC, 512], f32, tag="pssm",
                                                name="ps_num")
                        psbox["sst"] = psN.tile([128, 512], f32, tag="psst",
                                                name="ps_sst")
                    ps_num, ps_sst = psbox["num"], psbox["sst"]
                    tf = tfinp.tile([128, D], f32, tag="tf")
                    nc.vector.tensor_tensor(tf[:], tmid[i][:], ps_ap, ALU.add)
                    pr = prodp.tile([128, D], bf16, tag="prod")
                    nc.vector.tensor_tensor(pr[:], t_tiles[i][:], tf[:],
                                            ALU.mult)
                    nc.tensor.matmul(ps_num[:, :D], s_tiles[i][:], pr[:],
                                     start=(i == 0), stop=(i == NTT - 1))
                    pr2 = prodp.tile([128, D], bf16, tag="prod2")
                    nc.vector.tensor_tensor(pr2[:], tf[:], tf[:], ALU.mult)
                    nc.tensor.matmul(ps_sst[:SPC, :D], s_tiles[i][:], pr2[:],
                                     start=(i == 0), stop=(i == NTT - 1))
                    mcol = masks_sb[:, 4 * i + lb:4 * i + lb + 1]
                    if lb == 0:
                        nc.vector.tensor_scalar_mul(acc_tiles[i][:], tf[:],
                                                    mcol)
                    else:
                        nc.vector.scalar_tensor_tensor(acc_tiles[i][:], tf[:],
                                                       mcol, acc_tiles[i][:],
                                                       ALU.mult, ALU.add)
                    if lb == nlate - 1:
                        collect(fstat_box["s"], i, acc_tiles[i][:])

                with nc.named_scope(f"late{lb}"):
                    block(t_tiles, li, late_writer, tmid, carry["stat"])
                ps_num, ps_sst = psbox["num"], psbox["sst"]
                v8 = prodp.tile([SPC, D], f32, tag="v8", name="v8")
                nc.vector.tensor_tensor(v8[:], ssm[:], ps_sst[:SPC, :D],
                                        ALU.mult)
                v8b = prodp.tile([SPC, D], f32, tag="v8", name="v8")
                nc.scalar.activation(v8b[:], v8[:], AF.Sqrt)
                nc.vector.tensor_scalar_max(v8b[:], v8b[:], 1e-8)
                v8c = prodp.tile([SPC, D], f32, tag="v8", name="v8")
                nc.vector.reciprocal_approx_fast(v8c[:], v8b[:])
                v8d = prodp.tile([SPC, D], f32, tag="v8", name="v8")
                nc.vector.tensor_tensor(v8d[:], ps_num[:, :D], v8c[:],
                                        ALU.mult)
                qs = smallp.tile([SPC, 1], f32, tag="qs")
                nc.vector.tensor_reduce(qs[:], v8d[:], mybir.AxisListType.X,
                                        ALU.add)
                nc.vector.tensor_scalar_mul(cos_sb[:, lb:lb + 1], qs[:],
                                            1.0 / D)
            nc.vector.memset(cos_sb[:, 3:4], 0.0)
            nc.sync.dma_start(cos_d[:], cos_sb[:])
            final_in = acc_tiles
        else:
            final_in = t_tiles
            nc.vector.memset(cos_sb[:], 0.0)
            nc.sync.dma_start(cos_d[:], cos_sb[:])
            pro0 = smallp.tile([SPC, 4], f32, tag="pro")
            nc.vector.memset(pro0[:], 0.0)
            nc.sync.dma_start(pro_d[:], pro0[:])

        # ---------- final layernorm + output ----------
        if nlate > 0:
            statF = fstat_box["s"]
        else:
            statF = stats_pass(final_in, "lnstat")
        rstdF = finish_ln(statF)
        if not trivial_norm:
            gt = const.tile([128, D], f32, tag="ngt")
            bt = const.tile([128, D], f32, tag="nbt")
            nc.sync.dma_start(gt[:], normg_d[:])
            nc.sync.dma_start(bt[:], normb_d[:])
        for i in range(NTT):
            o = tfinp.tile([128, D], f32, tag="tf", name="tf")
            nc.vector.tensor_scalar(o[:], final_in[i][:],
                                    statF[:, 2 * i:2 * i + 1],
                                    rstdF[:, 2 * i + 1:2 * i + 2],
                                    ALU.subtract, ALU.mult)
            if not trivial_norm:
                nc.vector.tensor_tensor(o[:], o[:], gt[:], ALU.mult)
                nc.vector.tensor_tensor(o[:], o[:], bt[:], ALU.add)
            nc.sync.dma_start(out_d[i * 128:(i + 1) * 128, :], o[:])

    nc.compile()
    return nc


def _host_prep(inputs):
    import ml_dtypes
    bf = ml_dtypes.bfloat16
    f32 = np.float32

    conv_w = np.asarray(inputs["conv_w"], f32)
    conv_b = np.asarray(inputs["conv_b"], f32)
    pos_z = np.asarray(inputs["pos_z"], f32)[0]
    pos_x = np.asarray(inputs["pos_x"], f32)[0]
    ln1_g = np.asarray(inputs["ln1_g"], f32)
    ln1_b = np.asarray(inputs["ln1_b"], f32)
    ln2_g = np.asarray(inputs["ln2_g"], f32)
    ln2_b = np.asarray(inputs["ln2_b"], f32)
    qkv_w = np.asarray(inputs["qkv_w"], f32)
    qkv_b = np.asarray(inputs["qkv_b"], f32)
    proj_b = np.asarray(inputs["proj_b"], f32)
    fc1_w = np.asarray(inputs["fc1_w"], f32)
    fc1_b = np.asarray(inputs["fc1_b"], f32)
    fc2_b = np.asarray(inputs["fc2_b"], f32)

    wqkv = (ln1_g[:, :, None] * qkv_w).astype(bf)
    bqkv = (np.einsum("ld,ldo->lo", ln1_b, qkv_w) + qkv_b).astype(f32)
    wfc1 = (ln2_g[:, :, None] * fc1_w).astype(bf)
    bfc1 = (np.einsum("ld,ldo->lo", ln2_b, fc1_w) + fc1_b).astype(f32)

    zero_bias = not (np.any(bqkv) or np.any(proj_b) or np.any(bfc1)
                     or np.any(fc2_b) or np.any(np.asarray(inputs["mlp1_b"]))
                     or np.any(np.asarray(inputs["mlp2_b"])))

    pos_sample = np.concatenate([pos_z, pos_x], axis=0) + conv_b[None, :]
    pos_full = np.tile(pos_sample, (SPC, 1)).astype(f32)

    convw = np.ascontiguousarray(conv_w.reshape(D, 768).T).astype(bf)

    S = np.zeros((T, SPC), np.float32)
    S[np.arange(T), np.arange(T) // L] = 1.0

    mlp2w = np.zeros((160, 4), f32)
    mlp2w[:, :3] = np.asarray(inputs["mlp2_w"], f32)
    mlp2b = np.zeros((4,), f32)
    mlp2b[:3] = np.asarray(inputs["mlp2_b"], f32)

    norm_g = np.asarray(inputs["norm_g"], f32)
    norm_b = np.asarray(inputs["norm_b"], f32)
    trivial_norm = bool(np.all(norm_g == 1.0) and np.all(norm_b == 0.0))

    common = {
        "convw": convw,
        "pos": pos_full,
        "wqkv": np.ascontiguousarray(wqkv),
        "wproj": np.asarray(inputs["proj_w"], f32).astype(bf),
        "wfc1": np.ascontiguousarray(wfc1),
        "wfc2": np.asarray(inputs["fc2_w"], f32).astype(bf),
        "bqkv": bqkv, "bproj": proj_b, "bfc1": bfc1, "bfc2": fc2_b,
        "mlp1w": np.asarray(inputs["mlp1_w"], f32),
        "mlp2w": mlp2w,
        "mlp1b": np.asarray(inputs["mlp1_b"], f32),
        "mlp2b": mlp2b,
        "s_bf": S.astype(bf),
        "st_f": np.ascontiguousarray(S.T),
        "identb": np.eye(128, dtype=bf),
        "normg": np.tile(norm_g[None, :], (128, 1)).astype(f32),
        "normb": np.tile(norm_b[None, :], (128, 1)).astype(f32),
    }
    z = np.asarray(inputs["z"], f32)
    x = np.asarray(inputs["x"], f32)
    # im2col: patch features ordered (c, dy, dx) to match convw layout
    zp = z.reshape(B, 3, 8, 16, 8, 16).transpose(0, 2, 4, 1, 3, 5)
    zp = zp.reshape(B, 64, 768)
    xp = x.reshape(B, 3, 16, 16, 16, 16).transpose(0, 2, 4, 1, 3, 5)
    xp = xp.reshape(B, 256, 768)
    patches = np.concatenate([zp, xp], axis=1).astype(bf)  # [B, 320, 768]
    in_maps = []
    for c in range(NCORES):
        m = dict(common)
        m["patches"] = np.ascontiguousarray(
            patches[c * SPC:(c + 1) * SPC].reshape(T, 768))
        in_maps.append(m)
    return in_maps, zero_bias, trivial_norm


def kernel(**inputs):
    import os
    from concourse.bass_utils import run_bass_kernel_spmd

    ndense = int(os.environ.get("KERNEL_NDENSE", START + 1))
    nlate = int(os.environ.get("KERNEL_NLATE", NL - START - 1))

    in_maps, zero_bias, trivial_norm = _host_prep(inputs)
    key = (ndense, nlate, zero_bias, trivial_norm)
    if key not in _CACHE:
        _CACHE[key] = _build(*key)
    nc = _CACHE[key]

    trace = bool(int(os.environ.get("KERNEL_TRACE", "0")))
    res = run_bass_kernel_spmd(nc, in_maps, list(range(NCORES)), trace=trace)
    global LAST_EXEC_NS, LAST_SCOPES, LAST_INSTS
    LAST_EXEC_NS = res.exec_time_ns
    LAST_SCOPES = res.per_core_scope_times
    LAST_INSTS = res.instructions_and_trace
    outs, coss, pros = [], [], []
    for c in range(NCORES):
        r = res.results[c]
        outs.append(np.asarray(r["out"], np.float32).reshape(SPC, L, D))
        coss.append(np.asarray(r["cos"], np.float32)[:, :3])
        pros.append(np.asarray(r["pro"], np.float32)[:, :3])
    return (np.concatenate(outs, axis=0), np.concatenate(coss, axis=0),
            np.concatenate(pros, axis=0))
